# revision 25
# baseline (speedup 1.0000x reference)
"""GCN layer kernel for 8 Trainium2 NeuronCores.

Computes: out = relu(dinv[:,None] * ((adj+I).T @ (dinv[:,None] * (x@W))) + b)
where dinv = rsqrt(colsum(adj) + 1).

Strategy (MODE="dr2"): shard adj by COLUMNS across the 8 cores. Column
block c (together with the full scaled source features z) is exactly what's
needed to produce output rows [c*2048, (c+1)*2048) -- no device collectives.
The roofline is the adjacency stream: 16384^2 entries at the 1 byte/elem
minimum the PE can consume (fp8e4m3, entries {0,1,2} exact with the
self-loop folded in) = 33.5 MB/core at ~360 GB/s/core HBM ~= 90 us.

The PE runs at 1 output-row/cycle regardless of dtype (measured), so with
only F=64 features a plain z^T A matmul wastes half the 128-wide array and
becomes the bottleneck (~107 us). Fix: fp8 DoubleRow matmuls (k=256) with
the stationary packed as [z8 | r8] across the full 128 columns, where
z8 = fp8(z * 2^9) and r8 = fp8(residual). One instruction per moving
A-pair then computes z8^T A into PSUM partitions 0:64 AND r8^T A into
64:128 from a single pass of the moving tile: 256 matmuls x 512 cycles =
54.6 us, fully hidden under the DMA stream. The residual recovers bf16-
class accuracy (rel err 2.9e-3 vs the 3.6e-2 of raw fp8 z).

Epilogue per core: ACT copies ps[64:128] to SBUF (bf16), 4 exact identity
matmuls fold it back into ps[0:64] (the PE is the only cross-partition
mover; this avoids burning DMA-highway bytes), DVE multiplies by
dinv * 2^-9, ACT applies relu+bias, and the [64, 2048] result is written
back as bf16.
"""

import sys

import numpy as np

if "/opt/trn_rl_repo" not in sys.path:
    sys.path.insert(0, "/opt/trn_rl_repo")

import ml_dtypes

N = 16384
F = 64
NCORES = 8
NB = N // NCORES  # 2048 columns (= output rows) per core
P = 128
KT = N // P  # 128 k-tiles of 128 source rows each
MM_N = 512  # moving-operand free dim per matmul (one PSUM bank of f32)
DMA_BATCH = 2  # k-tiles per dma_start
APOOL_BUFS = 8  # in-flight A-tile slots (prefetch depth)
ALT_DMA = False  # alternate A-tile DMAs between the SP and ACT HWDGE rings
ZSCALE_P = 9  # z is scaled by 2^ZSCALE_P before fp8 quantization (dr mode)
MODE = "dr2"  # variant kernel() uses

_BASS_CACHE: dict = {}


def _build_bass(reps: int = 1, mode: str = "full"):
    """Build the per-core Bass module. reps>1 repeats the whole compute
    (same inputs/outputs) inside one NEFF -- used only for benchmarking
    device time independent of dispatch overhead. mode: "full" | "dma"
    (loads only, no matmul) | "mm" (matmuls from a single resident tile,
    1/128th of the DMA traffic)."""
    key = (reps, mode, DMA_BATCH, APOOL_BUFS, ALT_DMA)
    if key in _BASS_CACHE:
        return _BASS_CACHE[key]

    import concourse.mybir as mybir
    import concourse.tile as tile
    from concourse import bacc

    if mode in ("dr", "drdma", "drmm"):
        nc = _build_bass_dr(reps, mode)
        _BASS_CACHE[key] = nc
        return nc
    if mode in ("dr2", "dr2dma", "dr2mm"):
        nc = _build_bass_dr2(reps, mode)
        _BASS_CACHE[key] = nc
        return nc

    nc = bacc.Bacc("TRN2", target_bir_lowering=False, debug=False,
                   num_devices=NCORES)

    fp8 = mode in ("fp8", "fp8pair", "dma8", "mm8")
    pair = mode in ("pair", "fp8pair")
    a_dt = mybir.dt.float8e4 if fp8 else mybir.dt.bfloat16
    # pair mode: two col-group-tiled matmuls run concurrently. Each 512-chunk
    # nn gets its own PSUM bank (columns nn*512) with even chunks on
    # partitions 0-63 and odd chunks on 64-127, so no two accumulation
    # groups share a bank.
    b_p = 2 * F if pair else F       # bias/dinv partition count
    a_in = nc.dram_tensor("a", [N, NB], a_dt, kind="ExternalInput")
    z_in = nc.dram_tensor("z", [P, KT * F], mybir.dt.bfloat16,
                          kind="ExternalInput")
    b_in = nc.dram_tensor("bvec", [b_p, 1], mybir.dt.float32,
                          kind="ExternalInput")
    if fp8:
        d_in = nc.dram_tensor("dinv", [b_p, NB], mybir.dt.float32,
                              kind="ExternalInput")
    o_out = nc.dram_tensor("o", [F, NB], mybir.dt.float32,
                           kind="ExternalOutput")

    kb = DMA_BATCH
    # [KT/kb, 128, kb, NB]: group kb consecutive k-tiles into one DMA
    a_tiles = a_in.ap().rearrange("(g t p) i -> g p t i", t=kb, p=P)

    with tile.TileContext(nc) as tc:
        with (
            tc.tile_pool(name="singles", bufs=1) as singles,
            tc.tile_pool(name="apool", bufs=APOOL_BUFS) as apool,
            tc.tile_pool(name="psum", bufs=1, space="PSUM") as psum_pool,
        ):
            z_sb = singles.tile([P, KT * F], mybir.dt.bfloat16)
            nc.sync.dma_start(z_sb[:], z_in.ap())
            b_sb = singles.tile([b_p, 1], mybir.dt.float32)
            nc.sync.dma_start(b_sb[:], b_in.ap())
            d_sb = None
            if fp8:
                d_sb = singles.tile([b_p, NB], mybir.dt.float32, tag="d_sb")
                nc.sync.dma_start(d_sb[:], d_in.ap())

            mm_tile = None
            if mode in ("mm", "mm8"):
                mm_tile = singles.tile([P, kb, NB], a_dt, tag="mm_tile")
                nc.sync.dma_start(mm_tile[:], a_tiles[0])

            for _rep in range(reps):
                ps = psum_pool.tile([b_p, NB], mybir.dt.float32)

                for g in range(KT // kb):
                    if mode in ("mm", "mm8"):
                        at = mm_tile
                    else:
                        at = apool.tile([P, kb, NB], a_dt)
                        eng = nc.scalar if (ALT_DMA and g % 2) else nc.sync
                        eng.dma_start(at[:], a_tiles[g])
                    if mode in ("dma", "dma8"):
                        continue
                    for t in range(kb):
                        kt = g * kb + t
                        zk = z_sb[:, kt * F:(kt + 1) * F]
                        if pair:
                            for nn in range(NB // MM_N):
                                h = nn % 2
                                nc.tensor.matmul(
                                    ps[h * F:(h + 1) * F,
                                       nn * MM_N:(nn + 1) * MM_N],
                                    lhsT=zk,
                                    rhs=at[:, t, nn * MM_N:(nn + 1) * MM_N],
                                    start=(kt == 0),
                                    stop=(kt == KT - 1),
                                    tile_position=(0, h * F),
                                )
                        else:
                            for nn in range(NB // MM_N):
                                nc.tensor.matmul(
                                    ps[:, nn * MM_N:(nn + 1) * MM_N],
                                    lhsT=zk,
                                    rhs=at[:, t, nn * MM_N:(nn + 1) * MM_N],
                                    start=(kt == 0),
                                    stop=(kt == KT - 1),
                                )

                out_sb = singles.tile([b_p, NB], mybir.dt.float32,
                                      tag="out_sb")
                relu = mybir.ActivationFunctionType.Relu
                if mode in ("dma", "dma8"):
                    nc.vector.tensor_copy(out_sb[:F, :F], z_sb[:F, :F])
                    nc.sync.dma_start(o_out.ap(), out_sb[:F, :])
                elif pair:
                    # touch only the written PSUM quadrants
                    for nn in range(NB // MM_N):
                        h = nn % 2
                        sp = slice(h * F, (h + 1) * F)
                        sf = slice(nn * MM_N, (nn + 1) * MM_N)
                        if fp8:
                            nc.vector.tensor_mul(out_sb[sp, sf], ps[sp, sf],
                                                 d_sb[sp, sf])
                            nc.scalar.activation(out_sb[sp, sf],
                                                 out_sb[sp, sf], relu,
                                                 bias=b_sb[sp], scale=1.0)
                        else:
                            nc.scalar.activation(out_sb[sp, sf], ps[sp, sf],
                                                 relu, bias=b_sb[sp],
                                                 scale=1.0)
                        nc.sync.dma_start(o_out.ap()[:, sf], out_sb[sp, sf])
                elif fp8:
                    nc.vector.tensor_mul(out_sb[:], ps[:], d_sb[:])
                    nc.scalar.activation(out_sb[:], out_sb[:], relu,
                                         bias=b_sb[:], scale=1.0)
                    nc.sync.dma_start(o_out.ap(), out_sb[:])
                else:
                    nc.scalar.activation(out_sb[:], ps[:], relu,
                                         bias=b_sb[:], scale=1.0)
                    nc.sync.dma_start(o_out.ap(), out_sb[:])

    nc.compile()
    _BASS_CACHE[reps] = nc
    return nc


def _build_bass_dr(reps: int = 1, mode: str = "dr"):
    """DoubleRow fp8 variant: both operands fp8e4m3, 0.5 cycles/row.

    z is split as z8 + r8 (fp8 value + fp8-quantized residual, both scaled
    by 2^ZSCALE_P); each A tile [128, 2, NB] is consumed by two DoubleRow
    matmuls per 512-chunk (z8 then r8) accumulating into the same PSUM
    region, so the PE does 2 x 256-deep contractions per tile at 0.5
    cycles/row = 54.6us/pass, fully hidden under the ~94us A-stream DMA.
    The self-loop (+I) stays folded into A; dinv and the 2^-ZSCALE_P are
    applied on PSUM readout. mode: "dr" | "drdma" (loads only) | "drmm"
    (matmuls from one resident tile).
    """
    import concourse.mybir as mybir
    import concourse.tile as tile
    from concourse import bacc

    nc = bacc.Bacc("TRN2", target_bir_lowering=False, debug=False,
                   num_devices=NCORES)

    kb = DMA_BATCH
    assert kb % 2 == 0
    a_dt = mybir.dt.float8e4
    dr = mybir.MatmulPerfMode.DoubleRow

    a_in = nc.dram_tensor("a", [N, NB], a_dt, kind="ExternalInput")
    z8_in = nc.dram_tensor("z8", [P, KT * F], a_dt, kind="ExternalInput")
    r8_in = nc.dram_tensor("r8", [P, KT * F], a_dt, kind="ExternalInput")
    b_in = nc.dram_tensor("bvec", [F, 1], mybir.dt.float32,
                          kind="ExternalInput")
    d_in = nc.dram_tensor("dinv", [F, NB], mybir.dt.float32,
                          kind="ExternalInput")
    o_out = nc.dram_tensor("o", [F, NB], mybir.dt.float32,
                           kind="ExternalOutput")

    # [KT/kb, 128, kb, NB]: group kb consecutive k-tiles into one DMA
    a_tiles = a_in.ap().rearrange("(g t p) i -> g p t i", t=kb, p=P)
    NG = KT // kb          # a-tile groups
    NPAIR = kb // 2        # DoubleRow k-pair matmul groups per a-tile
    NCHUNK = NB // MM_N    # 512-wide output chunks

    with tile.TileContext(nc) as tc:
        with (
            tc.tile_pool(name="singles", bufs=1) as singles,
            tc.tile_pool(name="apool", bufs=APOOL_BUFS) as apool,
            tc.tile_pool(name="psum", bufs=2, space="PSUM") as psum_pool,
        ):
            z8_sb = singles.tile([P, KT, F], a_dt, tag="z8_sb")
            nc.sync.dma_start(z8_sb[:], z8_in.ap())
            r8_sb = singles.tile([P, KT, F], a_dt, tag="r8_sb")
            nc.sync.dma_start(r8_sb[:], r8_in.ap())
            b_sb = singles.tile([F, 1], mybir.dt.float32, tag="b_sb")
            nc.sync.dma_start(b_sb[:], b_in.ap())
            d_sb = singles.tile([F, NB], mybir.dt.float32, tag="d_sb")
            nc.sync.dma_start(d_sb[:], d_in.ap())

            mm_tile = None
            if mode == "drmm":
                mm_tile = singles.tile([P, kb, NB], a_dt, tag="mm_tile")
                nc.sync.dma_start(mm_tile[:], a_tiles[0])

            for _rep in range(reps):
                ps = psum_pool.tile([F, NB], mybir.dt.float32)

                for g in range(NG):
                    if mode == "drmm":
                        at = mm_tile
                    else:
                        at = apool.tile([P, kb, NB], a_dt)
                        eng = nc.scalar if (ALT_DMA and g % 2) else nc.sync
                        eng.dma_start(at[:], a_tiles[g])
                    if mode == "drdma":
                        continue
                    for t2 in range(NPAIR):
                        kt0 = (g * kb + 2 * t2) if mode != "drmm" else 2 * t2
                        first = g == 0 and t2 == 0
                        last = g == NG - 1 and t2 == NPAIR - 1
                        for nn in range(NCHUNK):
                            sf = slice(nn * MM_N, (nn + 1) * MM_N)
                            nc.tensor.matmul(
                                ps[:, sf],
                                lhsT=z8_sb[:, kt0:kt0 + 2, :],
                                rhs=at[:, 2 * t2:2 * t2 + 2, sf],
                                start=first,
                                stop=False,
                                perf_mode=dr,
                            )
                            nc.tensor.matmul(
                                ps[:, sf],
                                lhsT=r8_sb[:, kt0:kt0 + 2, :],
                                rhs=at[:, 2 * t2:2 * t2 + 2, sf],
                                start=False,
                                stop=last,
                                perf_mode=dr,
                            )

                out_sb = singles.tile([F, NB], mybir.dt.float32,
                                      tag="out_sb")
                relu = mybir.ActivationFunctionType.Relu
                if mode == "drdma":
                    nc.vector.tensor_copy(out_sb[:, :F], z8_sb[:F, 0, :])
                    nc.sync.dma_start(o_out.ap(), out_sb[:])
                else:
                    nc.vector.tensor_mul(out_sb[:], ps[:], d_sb[:])
                    nc.scalar.activation(out_sb[:], out_sb[:], relu,
                                         bias=b_sb[:], scale=1.0)
                    nc.sync.dma_start(o_out.ap(), out_sb[:])

    nc.compile()
    return nc


def _build_bass_dr2(reps: int = 1, mode: str = "dr2"):
    """DoubleRow with the full 128-wide stationary: [z8 | r8] packed in m.

    The PE runs at 1 output-row/cycle regardless of dtype (measured), so
    the only way to cut cycles is to do more per row. One DoubleRow matmul
    (k=256, m=128, n=512) computes z8^T A into PSUM partitions 0:64 AND
    r8^T A into 64:128 from a single pass of the moving A-tile: 256
    matmuls x 512 cycles = 54.6us/pass, fully hidden under the ~94us
    A-stream DMA. Epilogue: DMA ps[64:128] back to partitions 0:64, DVE
    add + dinv mul, ACT relu+bias.
    """
    import concourse.mybir as mybir
    import concourse.tile as tile
    from concourse import bacc

    nc = bacc.Bacc("TRN2", target_bir_lowering=False, debug=False,
                   num_devices=NCORES)

    kb = DMA_BATCH
    assert kb % 2 == 0
    a_dt = mybir.dt.float8e4
    dr = mybir.MatmulPerfMode.DoubleRow

    a_in = nc.dram_tensor("a", [N, NB], a_dt, kind="ExternalInput")
    zr_in = nc.dram_tensor("zr", [P, KT * 2 * F], a_dt, kind="ExternalInput")
    b_in = nc.dram_tensor("bvec", [F, 1], mybir.dt.float32,
                          kind="ExternalInput")
    d_in = nc.dram_tensor("dinv", [F, NB], mybir.dt.bfloat16,
                          kind="ExternalInput")
    id_in = nc.dram_tensor("ident", [2 * F, F], mybir.dt.bfloat16,
                           kind="ExternalInput")
    o_out = nc.dram_tensor("o", [F, NB], mybir.dt.bfloat16,
                           kind="ExternalOutput")

    a_tiles = a_in.ap().rearrange("(g t p) i -> g p t i", t=kb, p=P)
    NG = KT // kb
    NPAIR = kb // 2
    NCHUNK = NB // MM_N

    with tile.TileContext(nc) as tc:
        with (
            tc.tile_pool(name="singles", bufs=1) as singles,
            tc.tile_pool(name="apool", bufs=APOOL_BUFS) as apool,
            tc.tile_pool(name="psum", bufs=2, space="PSUM") as psum_pool,
        ):
            zr_sb = singles.tile([P, KT, 2 * F], a_dt, tag="zr_sb")
            nc.sync.dma_start(zr_sb[:], zr_in.ap())
            b_sb = singles.tile([F, 1], mybir.dt.float32, tag="b_sb")
            nc.sync.dma_start(b_sb[:], b_in.ap())
            d_sb = singles.tile([F, NB], mybir.dt.bfloat16, tag="d_sb")
            nc.sync.dma_start(d_sb[:], d_in.ap())
            id_sb = singles.tile([2 * F, F], mybir.dt.bfloat16, tag="id_sb")
            nc.sync.dma_start(id_sb[:], id_in.ap())

            mm_tile = None
            if mode == "dr2mm":
                mm_tile = singles.tile([P, kb, NB], a_dt, tag="mm_tile")
                nc.sync.dma_start(mm_tile[:], a_tiles[0])

            for _rep in range(reps):
                ps = psum_pool.tile([2 * F, NB], mybir.dt.float32)

                for g in range(NG):
                    if mode == "dr2mm":
                        at = mm_tile
                    else:
                        at = apool.tile([P, kb, NB], a_dt)
                        eng = nc.scalar if (ALT_DMA and g % 2) else nc.sync
                        eng.dma_start(at[:], a_tiles[g])
                    if mode == "dr2dma":
                        continue
                    for t2 in range(NPAIR):
                        kt0 = (g * kb + 2 * t2) if mode != "dr2mm" else 2 * t2
                        first = g == 0 and t2 == 0
                        last = g == NG - 1 and t2 == NPAIR - 1
                        for nn in range(NCHUNK):
                            sf = slice(nn * MM_N, (nn + 1) * MM_N)
                            nc.tensor.matmul(
                                ps[:, sf],
                                lhsT=zr_sb[:, kt0:kt0 + 2, :],
                                rhs=at[:, 2 * t2:2 * t2 + 2, sf],
                                start=first,
                                stop=last,
                                perf_mode=dr,
                            )

                out_sb = singles.tile([F, NB], mybir.dt.bfloat16,
                                      tag="out_sb")
                relu = mybir.ActivationFunctionType.Relu
                if mode == "dr2dma":
                    nc.vector.tensor_copy(out_sb[:, :F], zr_sb[:F, 0, :F])
                    nc.scalar.dma_start(o_out.ap(), out_sb[:])
                else:
                    # fold ps[64:128] (r8 half) into ps[0:64] via the PE:
                    # ACT copies the hi half to SBUF (partition-aligned),
                    # then 4 exact f32 identity matmuls accumulate it into
                    # the lo-half PSUM region -- no DMA-highway traffic.
                    hi128 = singles.tile([2 * F, NB], mybir.dt.bfloat16,
                                         tag="hi128")
                    nc.scalar.copy(hi128[F:2 * F, :], ps[F:2 * F, :])
                    for nn in range(NCHUNK):
                        sf = slice(nn * MM_N, (nn + 1) * MM_N)
                        nc.tensor.matmul(
                            ps[:F, sf],
                            lhsT=id_sb[F:2 * F, :],
                            rhs=hi128[F:2 * F, sf],
                            start=False,
                            stop=True,
                            skip_group_check=True,
                        )
                    nc.vector.tensor_mul(out_sb[:], ps[:F, :], d_sb[:])
                    nc.scalar.activation(out_sb[:], out_sb[:], relu,
                                         bias=b_sb[:], scale=1.0)
                    nc.scalar.dma_start(o_out.ap(), out_sb[:])

    nc.compile()
    return nc


def _host_prep(x, adj, W, b, mode=None):
    """Host-side sharding/preprocessing -> per-core input maps."""
    if mode is None:
        mode = MODE
    fp8 = mode in ("fp8", "fp8pair", "dma8", "mm8")
    pair = mode in ("pair", "fp8pair")
    x = np.asarray(x, dtype=np.float32)
    adj = np.asarray(adj, dtype=np.float32)
    W = np.asarray(W, dtype=np.float32)
    b = np.asarray(b, dtype=np.float32)

    deg = adj.sum(axis=0) + 1.0
    dinv = np.where(deg > 0, 1.0 / np.sqrt(deg), 0.0).astype(np.float32)

    z = (dinv[:, None] * (x @ W)).astype(np.float32)  # [N, F]

    if mode in ("dr", "drdma", "drmm", "dr2", "dr2dma", "dr2mm"):
        fp8 = ml_dtypes.float8_e4m3
        s = np.float32(2.0 ** ZSCALE_P)
        zs = z * s
        z8 = zs.astype(fp8)
        r8 = (zs - z8.astype(np.float32)).astype(fp8)

        def _kmajor(m):
            return np.ascontiguousarray(
                m.reshape(KT, P, F).transpose(1, 0, 2))  # [P, KT, F]

        z8_km = _kmajor(z8)
        r8_km = _kmajor(r8)
        b_dev = np.ascontiguousarray(b.reshape(F, 1))
        idx = np.arange(NB)
        in_maps = []
        for c in range(NCORES):
            cs = c * NB
            blk = adj[:, cs:cs + NB].copy()
            blk[cs + idx, idx] += 1.0  # self-loop (+I), exact in fp8
            dc = (dinv[cs:cs + NB] / s).astype(np.float32)
            d_dt = ml_dtypes.bfloat16 if mode.startswith("dr2") else np.float32
            m = {
                "a": blk.astype(fp8),
                "bvec": b_dev,
                "dinv": np.ascontiguousarray(
                    np.broadcast_to(dc, (F, NB)).astype(d_dt)),
            }
            if mode.startswith("dr2"):
                zr = np.concatenate([z8_km, r8_km], axis=2)  # [P, KT, 2F]
                m["zr"] = np.ascontiguousarray(zr.reshape(P, KT * 2 * F))
                ident = np.zeros((2 * F, F), ml_dtypes.bfloat16)
                ident[F + np.arange(F), np.arange(F)] = 1.0
                m["ident"] = ident
            else:
                m["z8"] = np.ascontiguousarray(z8_km.reshape(P, KT * F))
                m["r8"] = np.ascontiguousarray(r8_km.reshape(P, KT * F))
            in_maps.append(m)
        return in_maps
    # k-major layout: z_sb[p, kt*F + f] = z[kt*128 + p, f]
    z_dev = np.ascontiguousarray(
        z.reshape(KT, P, F).transpose(1, 0, 2).reshape(P, KT * F)
    ).astype(ml_dtypes.bfloat16)

    if pair:
        b_dev = np.ascontiguousarray(
            np.concatenate([b, b]).reshape(2 * F, 1))
    else:
        b_dev = np.ascontiguousarray(b.reshape(F, 1))

    def _pair_dinv(dc):
        # [128, NB]: chunk nn lives at [64*(nn%2):64*(nn%2+1), nn*512:...]
        d = np.zeros((2 * F, NB), np.float32)
        for nn in range(NB // MM_N):
            h = nn % 2
            d[h * F:(h + 1) * F, nn * MM_N:(nn + 1) * MM_N] = \
                dc[nn * MM_N:(nn + 1) * MM_N]
        return d

    in_maps = []
    idx = np.arange(NB)
    for c in range(NCORES):
        cs = c * NB
        if fp8:
            # adjacency stays exact {0,1,2} in fp8; dinv applied on device
            blk = adj[:, cs:cs + NB].copy()
            blk[cs + idx, idx] += 1.0  # self-loop (+I)
            dc = dinv[cs:cs + NB]
            m = {
                "a": blk.astype(ml_dtypes.float8_e4m3),
                "z": z_dev,
                "bvec": b_dev,
                "dinv": (_pair_dinv(dc) if pair else np.ascontiguousarray(
                    np.broadcast_to(dc, (F, NB)))),
            }
        else:
            blk = adj[:, cs:cs + NB] * dinv[cs:cs + NB][None, :]
            blk[cs + idx, idx] += dinv[cs + idx]  # fold self-loop (+I)
            m = {
                "a": blk.astype(ml_dtypes.bfloat16),
                "z": z_dev,
                "bvec": b_dev,
            }
        in_maps.append(m)
    return in_maps


def _assemble(results, mode=None):
    """Device outputs -> full [N, F] output."""
    if mode is None:
        mode = MODE
    out = np.empty((N, F), dtype=np.float32)
    for c in range(NCORES):
        out[c * NB:(c + 1) * NB, :] = results[c]["o"].T
    return out


def kernel(x, adj, W, b):
    from concourse import bass_utils

    nc = _build_bass(mode=MODE)
    in_maps = _host_prep(x, adj, W, b, mode=MODE)
    res = bass_utils.run_bass_kernel_spmd(nc, in_maps,
                                          core_ids=list(range(NCORES)))
    return _assemble(res.results, mode=MODE)



# revision 32
# speedup vs baseline: 1.0233x; 1.0233x over previous
"""GCN layer kernel for 8 Trainium2 NeuronCores.

Computes: out = relu(dinv[:,None] * ((adj+I).T @ (dinv[:,None] * (x@W))) + b)
where dinv = rsqrt(colsum(adj) + 1).

Strategy (MODE="dr2"): shard adj by COLUMNS across the 8 cores. Column
block c (together with the full scaled source features z) is exactly what's
needed to produce output rows [c*2048, (c+1)*2048) -- no device collectives.
The roofline is the adjacency stream: 16384^2 entries at the 1 byte/elem
minimum the PE can consume (fp8e4m3, entries {0,1,2} exact with the
self-loop folded in) = 33.5 MB/core at ~360 GB/s/core HBM ~= 90 us.

The PE runs at 1 output-row/cycle regardless of dtype (measured), so with
only F=64 features a plain z^T A matmul wastes half the 128-wide array and
becomes the bottleneck (~107 us). Fix: fp8 DoubleRow matmuls (k=256) with
the stationary packed as [z8 | r8] across the full 128 columns, where
z8 = fp8(z * 2^9) and r8 = fp8(residual). One instruction per moving
A-pair then computes z8^T A into PSUM partitions 0:64 AND r8^T A into
64:128 from a single pass of the moving tile: 256 matmuls x 512 cycles =
54.6 us, fully hidden under the DMA stream. The residual recovers bf16-
class accuracy (rel err 2.9e-3 vs the 3.6e-2 of raw fp8 z).

Epilogue per core: ACT copies ps[64:128] to SBUF (bf16), 4 exact identity
matmuls fold it back into ps[0:64] (the PE is the only cross-partition
mover; this avoids burning DMA-highway bytes), DVE multiplies by
dinv * 2^-9, ACT applies relu+bias, and the [64, 2048] result is written
back as bf16.
"""

import sys

import numpy as np

if "/opt/trn_rl_repo" not in sys.path:
    sys.path.insert(0, "/opt/trn_rl_repo")

import ml_dtypes

N = 16384
F = 64
NCORES = 8
NB = N // NCORES  # 2048 columns (= output rows) per core
P = 128
KT = N // P  # 128 k-tiles of 128 source rows each
MM_N = 512  # moving-operand free dim per matmul (one PSUM bank of f32)
DMA_BATCH = 2  # k-tiles per dma_start
APOOL_BUFS = 8  # in-flight A-tile slots (prefetch depth)
ALT_DMA = False  # alternate A-tile DMAs between the SP and ACT HWDGE rings
A_PRETILED = False  # host pre-tiles A so each DMA descriptor is kb*2048B
ZSCALE_P = 9  # z is scaled by 2^ZSCALE_P before fp8 quantization (dr mode)
MODE = "dr2"  # variant kernel() uses

_BASS_CACHE: dict = {}


def _build_bass(reps: int = 1, mode: str = "full"):
    """Build the per-core Bass module. reps>1 repeats the whole compute
    (same inputs/outputs) inside one NEFF -- used only for benchmarking
    device time independent of dispatch overhead. mode: "full" | "dma"
    (loads only, no matmul) | "mm" (matmuls from a single resident tile,
    1/128th of the DMA traffic)."""
    key = (reps, mode, DMA_BATCH, APOOL_BUFS, ALT_DMA, A_PRETILED)
    if key in _BASS_CACHE:
        return _BASS_CACHE[key]

    import concourse.mybir as mybir
    import concourse.tile as tile
    from concourse import bacc

    if mode in ("dr", "drdma", "drmm"):
        nc = _build_bass_dr(reps, mode)
        _BASS_CACHE[key] = nc
        return nc
    if mode in ("dr2", "dr2dma", "dr2mm"):
        nc = _build_bass_dr2(reps, mode)
        _BASS_CACHE[key] = nc
        return nc

    nc = bacc.Bacc("TRN2", target_bir_lowering=False, debug=False,
                   num_devices=NCORES)

    fp8 = mode in ("fp8", "fp8pair", "dma8", "mm8")
    pair = mode in ("pair", "fp8pair")
    a_dt = mybir.dt.float8e4 if fp8 else mybir.dt.bfloat16
    # pair mode: two col-group-tiled matmuls run concurrently. Each 512-chunk
    # nn gets its own PSUM bank (columns nn*512) with even chunks on
    # partitions 0-63 and odd chunks on 64-127, so no two accumulation
    # groups share a bank.
    b_p = 2 * F if pair else F       # bias/dinv partition count
    a_in = nc.dram_tensor("a", [N, NB], a_dt, kind="ExternalInput")
    z_in = nc.dram_tensor("z", [P, KT * F], mybir.dt.bfloat16,
                          kind="ExternalInput")
    b_in = nc.dram_tensor("bvec", [b_p, 1], mybir.dt.float32,
                          kind="ExternalInput")
    if fp8:
        d_in = nc.dram_tensor("dinv", [b_p, NB], mybir.dt.float32,
                              kind="ExternalInput")
    o_out = nc.dram_tensor("o", [F, NB], mybir.dt.float32,
                           kind="ExternalOutput")

    kb = DMA_BATCH
    # [KT/kb, 128, kb, NB]: group kb consecutive k-tiles into one DMA
    a_tiles = a_in.ap().rearrange("(g t p) i -> g p t i", t=kb, p=P)

    with tile.TileContext(nc) as tc:
        with (
            tc.tile_pool(name="singles", bufs=1) as singles,
            tc.tile_pool(name="apool", bufs=APOOL_BUFS) as apool,
            tc.tile_pool(name="psum", bufs=1, space="PSUM") as psum_pool,
        ):
            z_sb = singles.tile([P, KT * F], mybir.dt.bfloat16)
            nc.sync.dma_start(z_sb[:], z_in.ap())
            b_sb = singles.tile([b_p, 1], mybir.dt.float32)
            nc.sync.dma_start(b_sb[:], b_in.ap())
            d_sb = None
            if fp8:
                d_sb = singles.tile([b_p, NB], mybir.dt.float32, tag="d_sb")
                nc.sync.dma_start(d_sb[:], d_in.ap())

            mm_tile = None
            if mode in ("mm", "mm8"):
                mm_tile = singles.tile([P, kb, NB], a_dt, tag="mm_tile")
                nc.sync.dma_start(mm_tile[:], a_tiles[0])

            for _rep in range(reps):
                ps = psum_pool.tile([b_p, NB], mybir.dt.float32)

                for g in range(KT // kb):
                    if mode in ("mm", "mm8"):
                        at = mm_tile
                    else:
                        at = apool.tile([P, kb, NB], a_dt)
                        eng = nc.scalar if (ALT_DMA and g % 2) else nc.sync
                        eng.dma_start(at[:], a_tiles[g])
                    if mode in ("dma", "dma8"):
                        continue
                    for t in range(kb):
                        kt = g * kb + t
                        zk = z_sb[:, kt * F:(kt + 1) * F]
                        if pair:
                            for nn in range(NB // MM_N):
                                h = nn % 2
                                nc.tensor.matmul(
                                    ps[h * F:(h + 1) * F,
                                       nn * MM_N:(nn + 1) * MM_N],
                                    lhsT=zk,
                                    rhs=at[:, t, nn * MM_N:(nn + 1) * MM_N],
                                    start=(kt == 0),
                                    stop=(kt == KT - 1),
                                    tile_position=(0, h * F),
                                )
                        else:
                            for nn in range(NB // MM_N):
                                nc.tensor.matmul(
                                    ps[:, nn * MM_N:(nn + 1) * MM_N],
                                    lhsT=zk,
                                    rhs=at[:, t, nn * MM_N:(nn + 1) * MM_N],
                                    start=(kt == 0),
                                    stop=(kt == KT - 1),
                                )

                out_sb = singles.tile([b_p, NB], mybir.dt.float32,
                                      tag="out_sb")
                relu = mybir.ActivationFunctionType.Relu
                if mode in ("dma", "dma8"):
                    nc.vector.tensor_copy(out_sb[:F, :F], z_sb[:F, :F])
                    nc.sync.dma_start(o_out.ap(), out_sb[:F, :])
                elif pair:
                    # touch only the written PSUM quadrants
                    for nn in range(NB // MM_N):
                        h = nn % 2
                        sp = slice(h * F, (h + 1) * F)
                        sf = slice(nn * MM_N, (nn + 1) * MM_N)
                        if fp8:
                            nc.vector.tensor_mul(out_sb[sp, sf], ps[sp, sf],
                                                 d_sb[sp, sf])
                            nc.scalar.activation(out_sb[sp, sf],
                                                 out_sb[sp, sf], relu,
                                                 bias=b_sb[sp], scale=1.0)
                        else:
                            nc.scalar.activation(out_sb[sp, sf], ps[sp, sf],
                                                 relu, bias=b_sb[sp],
                                                 scale=1.0)
                        nc.sync.dma_start(o_out.ap()[:, sf], out_sb[sp, sf])
                elif fp8:
                    nc.vector.tensor_mul(out_sb[:], ps[:], d_sb[:])
                    nc.scalar.activation(out_sb[:], out_sb[:], relu,
                                         bias=b_sb[:], scale=1.0)
                    nc.sync.dma_start(o_out.ap(), out_sb[:])
                else:
                    nc.scalar.activation(out_sb[:], ps[:], relu,
                                         bias=b_sb[:], scale=1.0)
                    nc.sync.dma_start(o_out.ap(), out_sb[:])

    nc.compile()
    _BASS_CACHE[key] = nc
    return nc


def _build_bass_dr(reps: int = 1, mode: str = "dr"):
    """DoubleRow fp8 variant: both operands fp8e4m3, 0.5 cycles/row.

    z is split as z8 + r8 (fp8 value + fp8-quantized residual, both scaled
    by 2^ZSCALE_P); each A tile [128, 2, NB] is consumed by two DoubleRow
    matmuls per 512-chunk (z8 then r8) accumulating into the same PSUM
    region, so the PE does 2 x 256-deep contractions per tile at 0.5
    cycles/row = 54.6us/pass, fully hidden under the ~94us A-stream DMA.
    The self-loop (+I) stays folded into A; dinv and the 2^-ZSCALE_P are
    applied on PSUM readout. mode: "dr" | "drdma" (loads only) | "drmm"
    (matmuls from one resident tile).
    """
    import concourse.mybir as mybir
    import concourse.tile as tile
    from concourse import bacc

    nc = bacc.Bacc("TRN2", target_bir_lowering=False, debug=False,
                   num_devices=NCORES)

    kb = DMA_BATCH
    assert kb % 2 == 0
    a_dt = mybir.dt.float8e4
    dr = mybir.MatmulPerfMode.DoubleRow

    a_in = nc.dram_tensor("a", [N, NB], a_dt, kind="ExternalInput")
    z8_in = nc.dram_tensor("z8", [P, KT * F], a_dt, kind="ExternalInput")
    r8_in = nc.dram_tensor("r8", [P, KT * F], a_dt, kind="ExternalInput")
    b_in = nc.dram_tensor("bvec", [F, 1], mybir.dt.float32,
                          kind="ExternalInput")
    d_in = nc.dram_tensor("dinv", [F, NB], mybir.dt.float32,
                          kind="ExternalInput")
    o_out = nc.dram_tensor("o", [F, NB], mybir.dt.float32,
                           kind="ExternalOutput")

    # [KT/kb, 128, kb, NB]: group kb consecutive k-tiles into one DMA
    a_tiles = a_in.ap().rearrange("(g t p) i -> g p t i", t=kb, p=P)
    NG = KT // kb          # a-tile groups
    NPAIR = kb // 2        # DoubleRow k-pair matmul groups per a-tile
    NCHUNK = NB // MM_N    # 512-wide output chunks

    with tile.TileContext(nc) as tc:
        with (
            tc.tile_pool(name="singles", bufs=1) as singles,
            tc.tile_pool(name="apool", bufs=APOOL_BUFS) as apool,
            tc.tile_pool(name="psum", bufs=2, space="PSUM") as psum_pool,
        ):
            z8_sb = singles.tile([P, KT, F], a_dt, tag="z8_sb")
            nc.sync.dma_start(z8_sb[:], z8_in.ap())
            r8_sb = singles.tile([P, KT, F], a_dt, tag="r8_sb")
            nc.sync.dma_start(r8_sb[:], r8_in.ap())
            b_sb = singles.tile([F, 1], mybir.dt.float32, tag="b_sb")
            nc.sync.dma_start(b_sb[:], b_in.ap())
            d_sb = singles.tile([F, NB], mybir.dt.float32, tag="d_sb")
            nc.sync.dma_start(d_sb[:], d_in.ap())

            mm_tile = None
            if mode == "drmm":
                mm_tile = singles.tile([P, kb, NB], a_dt, tag="mm_tile")
                nc.sync.dma_start(mm_tile[:], a_tiles[0])

            for _rep in range(reps):
                ps = psum_pool.tile([F, NB], mybir.dt.float32)

                for g in range(NG):
                    if mode == "drmm":
                        at = mm_tile
                    else:
                        at = apool.tile([P, kb, NB], a_dt)
                        eng = nc.scalar if (ALT_DMA and g % 2) else nc.sync
                        eng.dma_start(at[:], a_tiles[g])
                    if mode == "drdma":
                        continue
                    for t2 in range(NPAIR):
                        kt0 = (g * kb + 2 * t2) if mode != "drmm" else 2 * t2
                        first = g == 0 and t2 == 0
                        last = g == NG - 1 and t2 == NPAIR - 1
                        for nn in range(NCHUNK):
                            sf = slice(nn * MM_N, (nn + 1) * MM_N)
                            nc.tensor.matmul(
                                ps[:, sf],
                                lhsT=z8_sb[:, kt0:kt0 + 2, :],
                                rhs=at[:, 2 * t2:2 * t2 + 2, sf],
                                start=first,
                                stop=False,
                                perf_mode=dr,
                            )
                            nc.tensor.matmul(
                                ps[:, sf],
                                lhsT=r8_sb[:, kt0:kt0 + 2, :],
                                rhs=at[:, 2 * t2:2 * t2 + 2, sf],
                                start=False,
                                stop=last,
                                perf_mode=dr,
                            )

                out_sb = singles.tile([F, NB], mybir.dt.float32,
                                      tag="out_sb")
                relu = mybir.ActivationFunctionType.Relu
                if mode == "drdma":
                    nc.vector.tensor_copy(out_sb[:, :F], z8_sb[:F, 0, :])
                    nc.sync.dma_start(o_out.ap(), out_sb[:])
                else:
                    nc.vector.tensor_mul(out_sb[:], ps[:], d_sb[:])
                    nc.scalar.activation(out_sb[:], out_sb[:], relu,
                                         bias=b_sb[:], scale=1.0)
                    nc.sync.dma_start(o_out.ap(), out_sb[:])

    nc.compile()
    return nc


def _build_bass_dr2(reps: int = 1, mode: str = "dr2"):
    """DoubleRow with the full 128-wide stationary: [z8 | r8] packed in m.

    The PE runs at 1 output-row/cycle regardless of dtype (measured), so
    the only way to cut cycles is to do more per row. One DoubleRow matmul
    (k=256, m=128, n=512) computes z8^T A into PSUM partitions 0:64 AND
    r8^T A into 64:128 from a single pass of the moving A-tile: 256
    matmuls x 512 cycles = 54.6us/pass, fully hidden under the ~94us
    A-stream DMA. Epilogue: DMA ps[64:128] back to partitions 0:64, DVE
    add + dinv mul, ACT relu+bias.
    """
    import concourse.mybir as mybir
    import concourse.tile as tile
    from concourse import bacc

    nc = bacc.Bacc("TRN2", target_bir_lowering=False, debug=False,
                   num_devices=NCORES)

    kb = DMA_BATCH
    assert kb % 2 == 0
    a_dt = mybir.dt.float8e4
    dr = mybir.MatmulPerfMode.DoubleRow

    if A_PRETILED:
        # host layout: row (g*128+p) holds the kb k-tile rows for partition
        # p of group g back to back -> one kb*2048B descriptor per partition
        a_in = nc.dram_tensor("a", [N // kb, kb * NB], a_dt,
                              kind="ExternalInput")
        a_tiles = a_in.ap().rearrange("(g p) (t i) -> g p t i", p=P, t=kb)
    else:
        a_in = nc.dram_tensor("a", [N, NB], a_dt, kind="ExternalInput")
        a_tiles = a_in.ap().rearrange("(g t p) i -> g p t i", t=kb, p=P)
    zr_in = nc.dram_tensor("zr", [P, KT * 2 * F], a_dt, kind="ExternalInput")
    b_in = nc.dram_tensor("bvec", [F, 1], mybir.dt.float32,
                          kind="ExternalInput")
    d_in = nc.dram_tensor("dinv", [F, NB], mybir.dt.bfloat16,
                          kind="ExternalInput")
    id_in = nc.dram_tensor("ident", [2 * F, F], mybir.dt.bfloat16,
                           kind="ExternalInput")
    o_out = nc.dram_tensor("o", [F, NB], mybir.dt.bfloat16,
                           kind="ExternalOutput")

    NG = KT // kb
    NPAIR = kb // 2
    NCHUNK = NB // MM_N

    with tile.TileContext(nc) as tc:
        with (
            tc.tile_pool(name="singles", bufs=1) as singles,
            tc.tile_pool(name="apool", bufs=APOOL_BUFS) as apool,
            tc.tile_pool(name="psum", bufs=2, space="PSUM") as psum_pool,
        ):
            zr_sb = singles.tile([P, KT, 2 * F], a_dt, tag="zr_sb")
            nc.sync.dma_start(zr_sb[:], zr_in.ap())
            b_sb = singles.tile([F, 1], mybir.dt.float32, tag="b_sb")
            nc.sync.dma_start(b_sb[:], b_in.ap())
            d_sb = singles.tile([F, NB], mybir.dt.bfloat16, tag="d_sb")
            nc.sync.dma_start(d_sb[:], d_in.ap())
            id_sb = singles.tile([2 * F, F], mybir.dt.bfloat16, tag="id_sb")
            nc.sync.dma_start(id_sb[:], id_in.ap())

            mm_tile = None
            if mode == "dr2mm":
                mm_tile = singles.tile([P, kb, NB], a_dt, tag="mm_tile")
                nc.sync.dma_start(mm_tile[:], a_tiles[0])

            for _rep in range(reps):
                ps = psum_pool.tile([2 * F, NB], mybir.dt.float32)

                for g in range(NG):
                    if mode == "dr2mm":
                        at = mm_tile
                    else:
                        at = apool.tile([P, kb, NB], a_dt)
                        eng = nc.scalar if (ALT_DMA and g % 2) else nc.sync
                        eng.dma_start(at[:], a_tiles[g])
                    if mode == "dr2dma":
                        continue
                    for t2 in range(NPAIR):
                        kt0 = (g * kb + 2 * t2) if mode != "dr2mm" else 2 * t2
                        first = g == 0 and t2 == 0
                        last = g == NG - 1 and t2 == NPAIR - 1
                        for nn in range(NCHUNK):
                            sf = slice(nn * MM_N, (nn + 1) * MM_N)
                            nc.tensor.matmul(
                                ps[:, sf],
                                lhsT=zr_sb[:, kt0:kt0 + 2, :],
                                rhs=at[:, 2 * t2:2 * t2 + 2, sf],
                                start=first,
                                stop=last,
                                perf_mode=dr,
                            )

                out_sb = singles.tile([F, NB], mybir.dt.bfloat16,
                                      tag="out_sb")
                relu = mybir.ActivationFunctionType.Relu
                if mode == "dr2dma":
                    nc.vector.tensor_copy(out_sb[:, :F], zr_sb[:F, 0, :F])
                    nc.scalar.dma_start(o_out.ap(), out_sb[:])
                else:
                    # fold ps[64:128] (r8 half) into ps[0:64] via the PE:
                    # ACT copies the hi half to SBUF (partition-aligned),
                    # then 4 exact f32 identity matmuls accumulate it into
                    # the lo-half PSUM region -- no DMA-highway traffic.
                    hi128 = singles.tile([2 * F, NB], mybir.dt.bfloat16,
                                         tag="hi128")
                    nc.scalar.copy(hi128[F:2 * F, :], ps[F:2 * F, :])
                    for nn in range(NCHUNK):
                        sf = slice(nn * MM_N, (nn + 1) * MM_N)
                        nc.tensor.matmul(
                            ps[:F, sf],
                            lhsT=id_sb[F:2 * F, :],
                            rhs=hi128[F:2 * F, sf],
                            start=False,
                            stop=True,
                            skip_group_check=True,
                        )
                    nc.vector.tensor_mul(out_sb[:], ps[:F, :], d_sb[:])
                    nc.scalar.activation(out_sb[:], out_sb[:], relu,
                                         bias=b_sb[:], scale=1.0)
                    nc.scalar.dma_start(o_out.ap(), out_sb[:])

    nc.compile()
    return nc


def _host_prep(x, adj, W, b, mode=None):
    """Host-side sharding/preprocessing -> per-core input maps."""
    if mode is None:
        mode = MODE
    fp8 = mode in ("fp8", "fp8pair", "dma8", "mm8")
    pair = mode in ("pair", "fp8pair")
    x = np.asarray(x, dtype=np.float32)
    adj = np.asarray(adj, dtype=np.float32)
    W = np.asarray(W, dtype=np.float32)
    b = np.asarray(b, dtype=np.float32)

    deg = adj.sum(axis=0) + 1.0
    dinv = np.where(deg > 0, 1.0 / np.sqrt(deg), 0.0).astype(np.float32)

    z = (dinv[:, None] * (x @ W)).astype(np.float32)  # [N, F]

    if mode in ("dr", "drdma", "drmm", "dr2", "dr2dma", "dr2mm"):
        fp8 = ml_dtypes.float8_e4m3
        s = np.float32(2.0 ** ZSCALE_P)
        zs = z * s
        z8 = zs.astype(fp8)
        r8 = (zs - z8.astype(np.float32)).astype(fp8)

        def _kmajor(m):
            return np.ascontiguousarray(
                m.reshape(KT, P, F).transpose(1, 0, 2))  # [P, KT, F]

        z8_km = _kmajor(z8)
        r8_km = _kmajor(r8)
        b_dev = np.ascontiguousarray(b.reshape(F, 1))
        idx = np.arange(NB)
        in_maps = []
        for c in range(NCORES):
            cs = c * NB
            blk = adj[:, cs:cs + NB].copy()
            blk[cs + idx, idx] += 1.0  # self-loop (+I), exact in fp8
            dc = (dinv[cs:cs + NB] / s).astype(np.float32)
            d_dt = ml_dtypes.bfloat16 if mode.startswith("dr2") else np.float32
            a_dev = blk.astype(fp8)
            if mode.startswith("dr2") and A_PRETILED:
                kb = DMA_BATCH
                a_dev = np.ascontiguousarray(
                    a_dev.reshape(KT // kb, kb, P, NB)
                    .transpose(0, 2, 1, 3).reshape(N // kb, kb * NB))
            m = {
                "a": a_dev,
                "bvec": b_dev,
                "dinv": np.ascontiguousarray(
                    np.broadcast_to(dc, (F, NB)).astype(d_dt)),
            }
            if mode.startswith("dr2"):
                zr = np.concatenate([z8_km, r8_km], axis=2)  # [P, KT, 2F]
                m["zr"] = np.ascontiguousarray(zr.reshape(P, KT * 2 * F))
                ident = np.zeros((2 * F, F), ml_dtypes.bfloat16)
                ident[F + np.arange(F), np.arange(F)] = 1.0
                m["ident"] = ident
            else:
                m["z8"] = np.ascontiguousarray(z8_km.reshape(P, KT * F))
                m["r8"] = np.ascontiguousarray(r8_km.reshape(P, KT * F))
            in_maps.append(m)
        return in_maps
    # k-major layout: z_sb[p, kt*F + f] = z[kt*128 + p, f]
    z_dev = np.ascontiguousarray(
        z.reshape(KT, P, F).transpose(1, 0, 2).reshape(P, KT * F)
    ).astype(ml_dtypes.bfloat16)

    if pair:
        b_dev = np.ascontiguousarray(
            np.concatenate([b, b]).reshape(2 * F, 1))
    else:
        b_dev = np.ascontiguousarray(b.reshape(F, 1))

    def _pair_dinv(dc):
        # [128, NB]: chunk nn lives at [64*(nn%2):64*(nn%2+1), nn*512:...]
        d = np.zeros((2 * F, NB), np.float32)
        for nn in range(NB // MM_N):
            h = nn % 2
            d[h * F:(h + 1) * F, nn * MM_N:(nn + 1) * MM_N] = \
                dc[nn * MM_N:(nn + 1) * MM_N]
        return d

    in_maps = []
    idx = np.arange(NB)
    for c in range(NCORES):
        cs = c * NB
        if fp8:
            # adjacency stays exact {0,1,2} in fp8; dinv applied on device
            blk = adj[:, cs:cs + NB].copy()
            blk[cs + idx, idx] += 1.0  # self-loop (+I)
            dc = dinv[cs:cs + NB]
            m = {
                "a": blk.astype(ml_dtypes.float8_e4m3),
                "z": z_dev,
                "bvec": b_dev,
                "dinv": (_pair_dinv(dc) if pair else np.ascontiguousarray(
                    np.broadcast_to(dc, (F, NB)))),
            }
        else:
            blk = adj[:, cs:cs + NB] * dinv[cs:cs + NB][None, :]
            blk[cs + idx, idx] += dinv[cs + idx]  # fold self-loop (+I)
            m = {
                "a": blk.astype(ml_dtypes.bfloat16),
                "z": z_dev,
                "bvec": b_dev,
            }
        in_maps.append(m)
    return in_maps


def _assemble(results, mode=None):
    """Device outputs -> full [N, F] output."""
    if mode is None:
        mode = MODE
    out = np.empty((N, F), dtype=np.float32)
    for c in range(NCORES):
        out[c * NB:(c + 1) * NB, :] = results[c]["o"].T
    return out


def kernel(x, adj, W, b):
    from concourse import bass_utils

    nc = _build_bass(mode=MODE)
    in_maps = _host_prep(x, adj, W, b, mode=MODE)
    res = bass_utils.run_bass_kernel_spmd(nc, in_maps,
                                          core_ids=list(range(NCORES)))
    return _assemble(res.results, mode=MODE)



# revision 45
# speedup vs baseline: 1.1981x; 1.1708x over previous
"""GCN layer kernel for 8 Trainium2 NeuronCores.

Computes: out = relu(dinv[:,None] * ((adj+I).T @ (dinv[:,None] * (x@W))) + b)
where dinv = rsqrt(colsum(adj) + 1).

Strategy (MODE="dr2"): shard adj by COLUMNS across the 8 cores. Column
block c (together with the full scaled source features z) is exactly what's
needed to produce output rows [c*2048, (c+1)*2048) -- no device collectives.
The roofline is the adjacency stream: 16384^2 entries at the 1 byte/elem
minimum the PE can consume (fp8e4m3, entries {0,1,2} exact with the
self-loop folded in) = 33.5 MB/core at ~360 GB/s/core HBM ~= 90 us.

The PE runs at 1 output-row/cycle regardless of dtype (measured), so with
only F=64 features a plain z^T A matmul wastes half the 128-wide array and
becomes the bottleneck (~107 us). Fix: fp8 DoubleRow matmuls (k=256) with
the stationary packed as [z8 | r8] across the full 128 columns, where
z8 = fp8(z * 2^9) and r8 = fp8(residual). One instruction per moving
A-pair then computes z8^T A into PSUM partitions 0:64 AND r8^T A into
64:128 from a single pass of the moving tile: 256 matmuls x 512 cycles =
54.6 us, fully hidden under the DMA stream. The residual recovers bf16-
class accuracy (rel err 2.9e-3 vs the 3.6e-2 of raw fp8 z).

Epilogue per core: ACT copies ps[64:128] to SBUF (bf16), 4 exact identity
matmuls fold it back into ps[0:64] (the PE is the only cross-partition
mover; this avoids burning DMA-highway bytes), DVE multiplies by
dinv * 2^-9, ACT applies relu+bias, and the [64, 2048] result is written
back as bf16.
"""

import sys

import numpy as np

if "/opt/trn_rl_repo" not in sys.path:
    sys.path.insert(0, "/opt/trn_rl_repo")

import ml_dtypes

N = 16384
F = 64
NCORES = 8
NB = N // NCORES  # 2048 columns (= output rows) per core
P = 128
KT = N // P  # 128 k-tiles of 128 source rows each
MM_N = 512  # moving-operand free dim per matmul (one PSUM bank of f32)
DMA_BATCH = 2  # k-tiles per dma_start
APOOL_BUFS = 14  # in-flight A-tile slots (prefetch depth)
ALT_DMA = False  # alternate A-tile DMAs between the SP and ACT HWDGE rings
A_PRETILED = False  # host pre-tiles A so each DMA descriptor is kb*2048B
NPACK = 28  # pk mode: tile-groups stored 2-entries/byte, decoded on ACT+DVE


def _pk_groups(ng):
    """Evenly spread NPACK packed groups among ng so decode (ACT+DVE)
    overlaps the plain-group DMA stream instead of phase-serializing."""
    return [g for g in range(ng)
            if (g * NPACK) // ng < ((g + 1) * NPACK) // ng]
ZSCALE_P = 9  # z is scaled by 2^ZSCALE_P before fp8 quantization (dr mode)
MODE = "pk"  # variant kernel() uses

_BASS_CACHE: dict = {}


def _build_bass(reps: int = 1, mode: str = "full"):
    """Build the per-core Bass module. reps>1 repeats the whole compute
    (same inputs/outputs) inside one NEFF -- used only for benchmarking
    device time independent of dispatch overhead. mode: "full" | "dma"
    (loads only, no matmul) | "mm" (matmuls from a single resident tile,
    1/128th of the DMA traffic)."""
    key = (reps, mode, DMA_BATCH, APOOL_BUFS, ALT_DMA, A_PRETILED)
    if key in _BASS_CACHE:
        return _BASS_CACHE[key]

    import concourse.mybir as mybir
    import concourse.tile as tile
    from concourse import bacc

    if mode in ("dr", "drdma", "drmm"):
        nc = _build_bass_dr(reps, mode)
        _BASS_CACHE[key] = nc
        return nc
    if mode in ("dr2", "dr2dma", "dr2mm"):
        nc = _build_bass_dr2(reps, mode)
        _BASS_CACHE[key] = nc
        return nc
    if mode == "pk":
        nc = _build_bass_pk(reps)
        _BASS_CACHE[key] = nc
        return nc

    nc = bacc.Bacc("TRN2", target_bir_lowering=False, debug=False,
                   num_devices=NCORES)

    fp8 = mode in ("fp8", "fp8pair", "dma8", "mm8")
    pair = mode in ("pair", "fp8pair")
    a_dt = mybir.dt.float8e4 if fp8 else mybir.dt.bfloat16
    # pair mode: two col-group-tiled matmuls run concurrently. Each 512-chunk
    # nn gets its own PSUM bank (columns nn*512) with even chunks on
    # partitions 0-63 and odd chunks on 64-127, so no two accumulation
    # groups share a bank.
    b_p = 2 * F if pair else F       # bias/dinv partition count
    a_in = nc.dram_tensor("a", [N, NB], a_dt, kind="ExternalInput")
    z_in = nc.dram_tensor("z", [P, KT * F], mybir.dt.bfloat16,
                          kind="ExternalInput")
    b_in = nc.dram_tensor("bvec", [b_p, 1], mybir.dt.float32,
                          kind="ExternalInput")
    if fp8:
        d_in = nc.dram_tensor("dinv", [b_p, NB], mybir.dt.float32,
                              kind="ExternalInput")
    o_out = nc.dram_tensor("o", [F, NB], mybir.dt.float32,
                           kind="ExternalOutput")

    kb = DMA_BATCH
    # [KT/kb, 128, kb, NB]: group kb consecutive k-tiles into one DMA
    a_tiles = a_in.ap().rearrange("(g t p) i -> g p t i", t=kb, p=P)

    with tile.TileContext(nc) as tc:
        with (
            tc.tile_pool(name="singles", bufs=1) as singles,
            tc.tile_pool(name="apool", bufs=APOOL_BUFS) as apool,
            tc.tile_pool(name="psum", bufs=1, space="PSUM") as psum_pool,
        ):
            z_sb = singles.tile([P, KT * F], mybir.dt.bfloat16)
            nc.sync.dma_start(z_sb[:], z_in.ap())
            b_sb = singles.tile([b_p, 1], mybir.dt.float32)
            nc.sync.dma_start(b_sb[:], b_in.ap())
            d_sb = None
            if fp8:
                d_sb = singles.tile([b_p, NB], mybir.dt.float32, tag="d_sb")
                nc.sync.dma_start(d_sb[:], d_in.ap())

            mm_tile = None
            if mode in ("mm", "mm8"):
                mm_tile = singles.tile([P, kb, NB], a_dt, tag="mm_tile")
                nc.sync.dma_start(mm_tile[:], a_tiles[0])

            for _rep in range(reps):
                ps = psum_pool.tile([b_p, NB], mybir.dt.float32)

                for g in range(KT // kb):
                    if mode in ("mm", "mm8"):
                        at = mm_tile
                    else:
                        at = apool.tile([P, kb, NB], a_dt)
                        eng = nc.scalar if (ALT_DMA and g % 2) else nc.sync
                        eng.dma_start(at[:], a_tiles[g])
                    if mode in ("dma", "dma8"):
                        continue
                    for t in range(kb):
                        kt = g * kb + t
                        zk = z_sb[:, kt * F:(kt + 1) * F]
                        if pair:
                            for nn in range(NB // MM_N):
                                h = nn % 2
                                nc.tensor.matmul(
                                    ps[h * F:(h + 1) * F,
                                       nn * MM_N:(nn + 1) * MM_N],
                                    lhsT=zk,
                                    rhs=at[:, t, nn * MM_N:(nn + 1) * MM_N],
                                    start=(kt == 0),
                                    stop=(kt == KT - 1),
                                    tile_position=(0, h * F),
                                )
                        else:
                            for nn in range(NB // MM_N):
                                nc.tensor.matmul(
                                    ps[:, nn * MM_N:(nn + 1) * MM_N],
                                    lhsT=zk,
                                    rhs=at[:, t, nn * MM_N:(nn + 1) * MM_N],
                                    start=(kt == 0),
                                    stop=(kt == KT - 1),
                                )

                out_sb = singles.tile([b_p, NB], mybir.dt.float32,
                                      tag="out_sb")
                relu = mybir.ActivationFunctionType.Relu
                if mode in ("dma", "dma8"):
                    nc.vector.tensor_copy(out_sb[:F, :F], z_sb[:F, :F])
                    nc.sync.dma_start(o_out.ap(), out_sb[:F, :])
                elif pair:
                    # touch only the written PSUM quadrants
                    for nn in range(NB // MM_N):
                        h = nn % 2
                        sp = slice(h * F, (h + 1) * F)
                        sf = slice(nn * MM_N, (nn + 1) * MM_N)
                        if fp8:
                            nc.vector.tensor_mul(out_sb[sp, sf], ps[sp, sf],
                                                 d_sb[sp, sf])
                            nc.scalar.activation(out_sb[sp, sf],
                                                 out_sb[sp, sf], relu,
                                                 bias=b_sb[sp], scale=1.0)
                        else:
                            nc.scalar.activation(out_sb[sp, sf], ps[sp, sf],
                                                 relu, bias=b_sb[sp],
                                                 scale=1.0)
                        nc.sync.dma_start(o_out.ap()[:, sf], out_sb[sp, sf])
                elif fp8:
                    nc.vector.tensor_mul(out_sb[:], ps[:], d_sb[:])
                    nc.scalar.activation(out_sb[:], out_sb[:], relu,
                                         bias=b_sb[:], scale=1.0)
                    nc.sync.dma_start(o_out.ap(), out_sb[:])
                else:
                    nc.scalar.activation(out_sb[:], ps[:], relu,
                                         bias=b_sb[:], scale=1.0)
                    nc.sync.dma_start(o_out.ap(), out_sb[:])

    nc.compile()
    _BASS_CACHE[key] = nc
    return nc


def _build_bass_dr(reps: int = 1, mode: str = "dr"):
    """DoubleRow fp8 variant: both operands fp8e4m3, 0.5 cycles/row.

    z is split as z8 + r8 (fp8 value + fp8-quantized residual, both scaled
    by 2^ZSCALE_P); each A tile [128, 2, NB] is consumed by two DoubleRow
    matmuls per 512-chunk (z8 then r8) accumulating into the same PSUM
    region, so the PE does 2 x 256-deep contractions per tile at 0.5
    cycles/row = 54.6us/pass, fully hidden under the ~94us A-stream DMA.
    The self-loop (+I) stays folded into A; dinv and the 2^-ZSCALE_P are
    applied on PSUM readout. mode: "dr" | "drdma" (loads only) | "drmm"
    (matmuls from one resident tile).
    """
    import concourse.mybir as mybir
    import concourse.tile as tile
    from concourse import bacc

    nc = bacc.Bacc("TRN2", target_bir_lowering=False, debug=False,
                   num_devices=NCORES)

    kb = DMA_BATCH
    assert kb % 2 == 0
    a_dt = mybir.dt.float8e4
    dr = mybir.MatmulPerfMode.DoubleRow

    a_in = nc.dram_tensor("a", [N, NB], a_dt, kind="ExternalInput")
    z8_in = nc.dram_tensor("z8", [P, KT * F], a_dt, kind="ExternalInput")
    r8_in = nc.dram_tensor("r8", [P, KT * F], a_dt, kind="ExternalInput")
    b_in = nc.dram_tensor("bvec", [F, 1], mybir.dt.float32,
                          kind="ExternalInput")
    d_in = nc.dram_tensor("dinv", [F, NB], mybir.dt.float32,
                          kind="ExternalInput")
    o_out = nc.dram_tensor("o", [F, NB], mybir.dt.float32,
                           kind="ExternalOutput")

    # [KT/kb, 128, kb, NB]: group kb consecutive k-tiles into one DMA
    a_tiles = a_in.ap().rearrange("(g t p) i -> g p t i", t=kb, p=P)
    NG = KT // kb          # a-tile groups
    NPAIR = kb // 2        # DoubleRow k-pair matmul groups per a-tile
    NCHUNK = NB // MM_N    # 512-wide output chunks

    with tile.TileContext(nc) as tc:
        with (
            tc.tile_pool(name="singles", bufs=1) as singles,
            tc.tile_pool(name="apool", bufs=APOOL_BUFS) as apool,
            tc.tile_pool(name="psum", bufs=2, space="PSUM") as psum_pool,
        ):
            z8_sb = singles.tile([P, KT, F], a_dt, tag="z8_sb")
            nc.sync.dma_start(z8_sb[:], z8_in.ap())
            r8_sb = singles.tile([P, KT, F], a_dt, tag="r8_sb")
            nc.sync.dma_start(r8_sb[:], r8_in.ap())
            b_sb = singles.tile([F, 1], mybir.dt.float32, tag="b_sb")
            nc.sync.dma_start(b_sb[:], b_in.ap())
            d_sb = singles.tile([F, NB], mybir.dt.float32, tag="d_sb")
            nc.sync.dma_start(d_sb[:], d_in.ap())

            mm_tile = None
            if mode == "drmm":
                mm_tile = singles.tile([P, kb, NB], a_dt, tag="mm_tile")
                nc.sync.dma_start(mm_tile[:], a_tiles[0])

            for _rep in range(reps):
                ps = psum_pool.tile([F, NB], mybir.dt.float32)

                for g in range(NG):
                    if mode == "drmm":
                        at = mm_tile
                    else:
                        at = apool.tile([P, kb, NB], a_dt)
                        eng = nc.scalar if (ALT_DMA and g % 2) else nc.sync
                        eng.dma_start(at[:], a_tiles[g])
                    if mode == "drdma":
                        continue
                    for t2 in range(NPAIR):
                        kt0 = (g * kb + 2 * t2) if mode != "drmm" else 2 * t2
                        first = g == 0 and t2 == 0
                        last = g == NG - 1 and t2 == NPAIR - 1
                        for nn in range(NCHUNK):
                            sf = slice(nn * MM_N, (nn + 1) * MM_N)
                            nc.tensor.matmul(
                                ps[:, sf],
                                lhsT=z8_sb[:, kt0:kt0 + 2, :],
                                rhs=at[:, 2 * t2:2 * t2 + 2, sf],
                                start=first,
                                stop=False,
                                perf_mode=dr,
                            )
                            nc.tensor.matmul(
                                ps[:, sf],
                                lhsT=r8_sb[:, kt0:kt0 + 2, :],
                                rhs=at[:, 2 * t2:2 * t2 + 2, sf],
                                start=False,
                                stop=last,
                                perf_mode=dr,
                            )

                out_sb = singles.tile([F, NB], mybir.dt.float32,
                                      tag="out_sb")
                relu = mybir.ActivationFunctionType.Relu
                if mode == "drdma":
                    nc.vector.tensor_copy(out_sb[:, :F], z8_sb[:F, 0, :])
                    nc.sync.dma_start(o_out.ap(), out_sb[:])
                else:
                    nc.vector.tensor_mul(out_sb[:], ps[:], d_sb[:])
                    nc.scalar.activation(out_sb[:], out_sb[:], relu,
                                         bias=b_sb[:], scale=1.0)
                    nc.sync.dma_start(o_out.ap(), out_sb[:])

    nc.compile()
    return nc


def _build_bass_dr2(reps: int = 1, mode: str = "dr2"):
    """DoubleRow with the full 128-wide stationary: [z8 | r8] packed in m.

    The PE runs at 1 output-row/cycle regardless of dtype (measured), so
    the only way to cut cycles is to do more per row. One DoubleRow matmul
    (k=256, m=128, n=512) computes z8^T A into PSUM partitions 0:64 AND
    r8^T A into 64:128 from a single pass of the moving A-tile: 256
    matmuls x 512 cycles = 54.6us/pass, fully hidden under the ~94us
    A-stream DMA. Epilogue: DMA ps[64:128] back to partitions 0:64, DVE
    add + dinv mul, ACT relu+bias.
    """
    import concourse.mybir as mybir
    import concourse.tile as tile
    from concourse import bacc

    nc = bacc.Bacc("TRN2", target_bir_lowering=False, debug=False,
                   num_devices=NCORES)

    kb = DMA_BATCH
    assert kb % 2 == 0
    a_dt = mybir.dt.float8e4
    dr = mybir.MatmulPerfMode.DoubleRow

    if A_PRETILED:
        # host layout: row (g*128+p) holds the kb k-tile rows for partition
        # p of group g back to back -> one kb*2048B descriptor per partition
        a_in = nc.dram_tensor("a", [N // kb, kb * NB], a_dt,
                              kind="ExternalInput")
        a_tiles = a_in.ap().rearrange("(g p) (t i) -> g p t i", p=P, t=kb)
    else:
        a_in = nc.dram_tensor("a", [N, NB], a_dt, kind="ExternalInput")
        a_tiles = a_in.ap().rearrange("(g t p) i -> g p t i", t=kb, p=P)
    zr_in = nc.dram_tensor("zr", [P, KT * 2 * F], a_dt, kind="ExternalInput")
    b_in = nc.dram_tensor("bvec", [F, 1], mybir.dt.float32,
                          kind="ExternalInput")
    d_in = nc.dram_tensor("dinv", [F, NB], mybir.dt.bfloat16,
                          kind="ExternalInput")
    id_in = nc.dram_tensor("ident", [2 * F, F], mybir.dt.bfloat16,
                           kind="ExternalInput")
    o_out = nc.dram_tensor("o", [F, NB], mybir.dt.bfloat16,
                           kind="ExternalOutput")

    NG = KT // kb
    NPAIR = kb // 2
    NCHUNK = NB // MM_N

    with tile.TileContext(nc) as tc:
        with (
            tc.tile_pool(name="singles", bufs=1) as singles,
            tc.tile_pool(name="apool", bufs=APOOL_BUFS) as apool,
            tc.tile_pool(name="psum", bufs=2, space="PSUM") as psum_pool,
        ):
            zr_sb = singles.tile([P, KT, 2 * F], a_dt, tag="zr_sb")
            nc.sync.dma_start(zr_sb[:], zr_in.ap())
            b_sb = singles.tile([F, 1], mybir.dt.float32, tag="b_sb")
            nc.sync.dma_start(b_sb[:], b_in.ap())
            d_sb = singles.tile([F, NB], mybir.dt.bfloat16, tag="d_sb")
            nc.sync.dma_start(d_sb[:], d_in.ap())
            id_sb = singles.tile([2 * F, F], mybir.dt.bfloat16, tag="id_sb")
            nc.sync.dma_start(id_sb[:], id_in.ap())

            mm_tile = None
            if mode == "dr2mm":
                mm_tile = singles.tile([P, kb, NB], a_dt, tag="mm_tile")
                nc.sync.dma_start(mm_tile[:], a_tiles[0])

            for _rep in range(reps):
                ps = psum_pool.tile([2 * F, NB], mybir.dt.float32)

                for g in range(NG):
                    if mode == "dr2mm":
                        at = mm_tile
                    else:
                        at = apool.tile([P, kb, NB], a_dt)
                        eng = nc.scalar if (ALT_DMA and g % 2) else nc.sync
                        eng.dma_start(at[:], a_tiles[g])
                    if mode == "dr2dma":
                        continue
                    for t2 in range(NPAIR):
                        kt0 = (g * kb + 2 * t2) if mode != "dr2mm" else 2 * t2
                        first = g == 0 and t2 == 0
                        last = g == NG - 1 and t2 == NPAIR - 1
                        for nn in range(NCHUNK):
                            sf = slice(nn * MM_N, (nn + 1) * MM_N)
                            nc.tensor.matmul(
                                ps[:, sf],
                                lhsT=zr_sb[:, kt0:kt0 + 2, :],
                                rhs=at[:, 2 * t2:2 * t2 + 2, sf],
                                start=first,
                                stop=last,
                                perf_mode=dr,
                            )

                out_sb = singles.tile([F, NB], mybir.dt.bfloat16,
                                      tag="out_sb")
                relu = mybir.ActivationFunctionType.Relu
                if mode == "dr2dma":
                    nc.vector.tensor_copy(out_sb[:, :F], zr_sb[:F, 0, :F])
                    nc.scalar.dma_start(o_out.ap(), out_sb[:])
                else:
                    # fold ps[64:128] (r8 half) into ps[0:64] via the PE:
                    # ACT copies the hi half to SBUF (partition-aligned),
                    # then 4 exact f32 identity matmuls accumulate it into
                    # the lo-half PSUM region -- no DMA-highway traffic.
                    hi128 = singles.tile([2 * F, NB], mybir.dt.bfloat16,
                                         tag="hi128")
                    nc.scalar.copy(hi128[F:2 * F, :], ps[F:2 * F, :])
                    for nn in range(NCHUNK):
                        sf = slice(nn * MM_N, (nn + 1) * MM_N)
                        nc.tensor.matmul(
                            ps[:F, sf],
                            lhsT=id_sb[F:2 * F, :],
                            rhs=hi128[F:2 * F, sf],
                            start=False,
                            stop=True,
                            skip_group_check=True,
                        )
                    nc.vector.tensor_mul(out_sb[:], ps[:F, :], d_sb[:])
                    nc.scalar.activation(out_sb[:], out_sb[:], relu,
                                         bias=b_sb[:], scale=1.0)
                    nc.scalar.dma_start(o_out.ap(), out_sb[:])

    nc.compile()
    return nc


def _build_bass_pk(reps: int = 1):
    """dr2 + sign-domain packed A for NPACK of the 64 tile-groups.

    Packed byte v = (2a-1) + 0.125*(2b-1) (exact fp8) holds two k-rows per
    byte, halving HBM bytes for that fraction. Decode is one op per engine:
    sa = Sign(v) on ACT, t' = sa - v on DVE, written straight into the
    moving tile's two k-slices. The matmul consumes sa/t' with stationary
    slices pre-scaled (x0.5 for sign-slices, x-4 for t'-slices); the rank-1
    offsets (0.5 * colsum of z over packed rows) and the self-loop +z are
    folded into one zown tensor added to PSUM before the dinv multiply.
    NPACK balances DMA savings against ACT/DVE decode throughput.
    """
    import concourse.mybir as mybir
    import concourse.tile as tile
    from concourse import bacc

    nc = bacc.Bacc("TRN2", target_bir_lowering=False, debug=False,
                   num_devices=NCORES)

    kb = 2
    a_dt = mybir.dt.float8e4
    dr = mybir.MatmulPerfMode.DoubleRow
    NG = KT // kb
    NCHUNK = NB // MM_N
    npk = NPACK
    assert 0 <= npk <= NG
    pkset = set(_pk_groups(NG))
    assert len(pkset) == npk

    a_in = nc.dram_tensor("a", [N - npk * 2 * P, NB], a_dt,
                          kind="ExternalInput")
    apk_in = nc.dram_tensor("apk", [npk * P, NB], a_dt,
                            kind="ExternalInput")
    zr_in = nc.dram_tensor("zr", [P, KT * 2 * F], a_dt, kind="ExternalInput")
    b_in = nc.dram_tensor("bvec", [F, 1], mybir.dt.float32,
                          kind="ExternalInput")
    d_in = nc.dram_tensor("dinv", [F, NB], mybir.dt.bfloat16,
                          kind="ExternalInput")
    zown_in = nc.dram_tensor("zown", [F, NB], mybir.dt.float32,
                             kind="ExternalInput")
    id_in = nc.dram_tensor("ident", [2 * F, F], mybir.dt.bfloat16,
                           kind="ExternalInput")
    o_out = nc.dram_tensor("o", [F, NB], mybir.dt.bfloat16,
                           kind="ExternalOutput")

    a_tiles = a_in.ap().rearrange("(g t p) i -> g p t i", t=kb, p=P)
    pk_tiles = apk_in.ap().rearrange("(g p) i -> g p i", p=P)

    with tile.TileContext(nc) as tc:
        with (
            tc.tile_pool(name="singles", bufs=1) as singles,
            tc.tile_pool(name="apool", bufs=APOOL_BUFS) as apool,
            tc.tile_pool(name="pkpool", bufs=6) as pkpool,
            tc.tile_pool(name="psum", bufs=2, space="PSUM") as psum_pool,
        ):
            zr_sb = singles.tile([P, KT, 2 * F], a_dt, tag="zr_sb")
            nc.sync.dma_start(zr_sb[:], zr_in.ap())
            b_sb = singles.tile([F, 1], mybir.dt.float32, tag="b_sb")
            nc.sync.dma_start(b_sb[:], b_in.ap())
            d_sb = singles.tile([F, NB], mybir.dt.bfloat16, tag="d_sb")
            nc.sync.dma_start(d_sb[:], d_in.ap())
            zown_sb = singles.tile([F, NB], mybir.dt.float32, tag="zown_sb")
            nc.sync.dma_start(zown_sb[:], zown_in.ap())
            id_sb = singles.tile([2 * F, F], mybir.dt.bfloat16, tag="id_sb")
            nc.sync.dma_start(id_sb[:], id_in.ap())

            for _rep in range(reps):
                ps = psum_pool.tile([2 * F, NB], mybir.dt.float32)

                pk_idx = 0
                pl_idx = 0
                for g in range(NG):
                    at = apool.tile([P, kb, NB], a_dt)
                    if g in pkset:
                        vt = pkpool.tile([P, NB], a_dt)
                        nc.sync.dma_start(vt[:], pk_tiles[pk_idx])
                        pk_idx += 1
                        nc.scalar.sign(at[:, 0, :], vt[:])
                        nc.vector.tensor_sub(at[:, 1, :], at[:, 0, :],
                                             vt[:])
                    else:
                        nc.sync.dma_start(at[:], a_tiles[pl_idx])
                        pl_idx += 1
                    kt0 = 2 * g
                    for nn in range(NCHUNK):
                        sf = slice(nn * MM_N, (nn + 1) * MM_N)
                        nc.tensor.matmul(
                            ps[:, sf],
                            lhsT=zr_sb[:, kt0:kt0 + 2, :],
                            rhs=at[:, :, sf],
                            start=(g == 0),
                            stop=(g == NG - 1),
                            perf_mode=dr,
                        )

                out_sb = singles.tile([F, NB], mybir.dt.bfloat16,
                                      tag="out_sb")
                relu = mybir.ActivationFunctionType.Relu
                hi128 = singles.tile([2 * F, NB], mybir.dt.bfloat16,
                                     tag="hi128")
                nc.scalar.copy(hi128[F:2 * F, :], ps[F:2 * F, :])
                for nn in range(NCHUNK):
                    sf = slice(nn * MM_N, (nn + 1) * MM_N)
                    nc.tensor.matmul(
                        ps[:F, sf],
                        lhsT=id_sb[F:2 * F, :],
                        rhs=hi128[F:2 * F, sf],
                        start=False,
                        stop=True,
                        skip_group_check=True,
                    )
                tmp_sb = singles.tile([F, NB], mybir.dt.float32,
                                      tag="tmp_sb")
                nc.vector.tensor_add(tmp_sb[:], ps[:F, :], zown_sb[:])
                nc.vector.tensor_mul(out_sb[:], tmp_sb[:], d_sb[:])
                nc.scalar.activation(out_sb[:], out_sb[:], relu,
                                     bias=b_sb[:], scale=1.0)
                nc.scalar.dma_start(o_out.ap(), out_sb[:])

    nc.compile()
    return nc


def _host_prep(x, adj, W, b, mode=None):
    """Host-side sharding/preprocessing -> per-core input maps."""
    if mode is None:
        mode = MODE
    fp8 = mode in ("fp8", "fp8pair", "dma8", "mm8")
    pair = mode in ("pair", "fp8pair")
    x = np.asarray(x, dtype=np.float32)
    adj = np.asarray(adj, dtype=np.float32)
    W = np.asarray(W, dtype=np.float32)
    b = np.asarray(b, dtype=np.float32)

    deg = adj.sum(axis=0) + 1.0
    dinv = np.where(deg > 0, 1.0 / np.sqrt(deg), 0.0).astype(np.float32)

    z = (dinv[:, None] * (x @ W)).astype(np.float32)  # [N, F]

    if mode == "pk":
        fp8 = ml_dtypes.float8_e4m3
        s = np.float32(2.0 ** ZSCALE_P)
        zs = z * s
        z8 = zs.astype(fp8)
        z8f = z8.astype(np.float32)
        r8 = (zs - z8f).astype(fp8)
        r8f = r8.astype(np.float32)
        npk = NPACK
        pklist = _pk_groups(KT // 2)
        pkset = set(pklist)
        pllist = [g for g in range(KT // 2) if g not in pkset]

        fac = np.ones(KT, np.float32)
        for g in pklist:
            fac[2 * g] = 0.5
            fac[2 * g + 1] = -4.0
        rowfac = np.repeat(fac, P)[:, None]

        def _kmajor(m):
            return np.ascontiguousarray(
                m.reshape(KT, P, F).transpose(1, 0, 2))

        zr = np.concatenate(
            [_kmajor((z8f * rowfac).astype(fp8)),
             _kmajor((r8f * rowfac).astype(fp8))], axis=2)
        zr_dev = np.ascontiguousarray(zr.reshape(P, KT * 2 * F))
        zsum = z8f + r8f
        pkrows = np.zeros(N, bool)
        for g in pklist:
            pkrows[g * 2 * P:(g + 1) * 2 * P] = True
        corr = 0.5 * zsum[pkrows, :].astype(np.float64).sum(
            axis=0).astype(np.float32)  # [F]
        b_dev = np.ascontiguousarray(b.reshape(F, 1))
        ident = np.zeros((2 * F, F), ml_dtypes.bfloat16)
        ident[F + np.arange(F), np.arange(F)] = 1.0
        in_maps = []
        for c in range(NCORES):
            cs = c * NB
            blk = adj[:, cs:cs + NB]  # raw {0,1}; self-loop via zown
            A4 = blk.reshape(KT // 2, 2, P, NB)
            Apk = A4[pklist]  # [npk, 2, P, NB]
            v = (2.0 * Apk[:, 0] - 1.0) + 0.125 * (2.0 * Apk[:, 1] - 1.0)
            Apl = A4[pllist].reshape(-1, NB)  # plain groups, (g t p) order
            dc = (dinv[cs:cs + NB] / s).astype(np.float32)
            zown = zsum[cs:cs + NB, :].T + corr[:, None]  # [F, NB]
            in_maps.append({
                "a": np.ascontiguousarray(Apl).astype(fp8),
                "apk": np.ascontiguousarray(
                    v.reshape(npk * P, NB)).astype(fp8),
                "zr": zr_dev,
                "bvec": b_dev,
                "dinv": np.ascontiguousarray(
                    np.broadcast_to(dc, (F, NB)).astype(ml_dtypes.bfloat16)),
                "zown": np.ascontiguousarray(zown.astype(np.float32)),
                "ident": ident,
            })
        return in_maps

    if mode in ("dr", "drdma", "drmm", "dr2", "dr2dma", "dr2mm"):
        fp8 = ml_dtypes.float8_e4m3
        s = np.float32(2.0 ** ZSCALE_P)
        zs = z * s
        z8 = zs.astype(fp8)
        r8 = (zs - z8.astype(np.float32)).astype(fp8)

        def _kmajor(m):
            return np.ascontiguousarray(
                m.reshape(KT, P, F).transpose(1, 0, 2))  # [P, KT, F]

        z8_km = _kmajor(z8)
        r8_km = _kmajor(r8)
        b_dev = np.ascontiguousarray(b.reshape(F, 1))
        idx = np.arange(NB)
        in_maps = []
        for c in range(NCORES):
            cs = c * NB
            blk = adj[:, cs:cs + NB].copy()
            blk[cs + idx, idx] += 1.0  # self-loop (+I), exact in fp8
            dc = (dinv[cs:cs + NB] / s).astype(np.float32)
            d_dt = ml_dtypes.bfloat16 if mode.startswith("dr2") else np.float32
            a_dev = blk.astype(fp8)
            if mode.startswith("dr2") and A_PRETILED:
                kb = DMA_BATCH
                a_dev = np.ascontiguousarray(
                    a_dev.reshape(KT // kb, kb, P, NB)
                    .transpose(0, 2, 1, 3).reshape(N // kb, kb * NB))
            m = {
                "a": a_dev,
                "bvec": b_dev,
                "dinv": np.ascontiguousarray(
                    np.broadcast_to(dc, (F, NB)).astype(d_dt)),
            }
            if mode.startswith("dr2"):
                zr = np.concatenate([z8_km, r8_km], axis=2)  # [P, KT, 2F]
                m["zr"] = np.ascontiguousarray(zr.reshape(P, KT * 2 * F))
                ident = np.zeros((2 * F, F), ml_dtypes.bfloat16)
                ident[F + np.arange(F), np.arange(F)] = 1.0
                m["ident"] = ident
            else:
                m["z8"] = np.ascontiguousarray(z8_km.reshape(P, KT * F))
                m["r8"] = np.ascontiguousarray(r8_km.reshape(P, KT * F))
            in_maps.append(m)
        return in_maps
    # k-major layout: z_sb[p, kt*F + f] = z[kt*128 + p, f]
    z_dev = np.ascontiguousarray(
        z.reshape(KT, P, F).transpose(1, 0, 2).reshape(P, KT * F)
    ).astype(ml_dtypes.bfloat16)

    if pair:
        b_dev = np.ascontiguousarray(
            np.concatenate([b, b]).reshape(2 * F, 1))
    else:
        b_dev = np.ascontiguousarray(b.reshape(F, 1))

    def _pair_dinv(dc):
        # [128, NB]: chunk nn lives at [64*(nn%2):64*(nn%2+1), nn*512:...]
        d = np.zeros((2 * F, NB), np.float32)
        for nn in range(NB // MM_N):
            h = nn % 2
            d[h * F:(h + 1) * F, nn * MM_N:(nn + 1) * MM_N] = \
                dc[nn * MM_N:(nn + 1) * MM_N]
        return d

    in_maps = []
    idx = np.arange(NB)
    for c in range(NCORES):
        cs = c * NB
        if fp8:
            # adjacency stays exact {0,1,2} in fp8; dinv applied on device
            blk = adj[:, cs:cs + NB].copy()
            blk[cs + idx, idx] += 1.0  # self-loop (+I)
            dc = dinv[cs:cs + NB]
            m = {
                "a": blk.astype(ml_dtypes.float8_e4m3),
                "z": z_dev,
                "bvec": b_dev,
                "dinv": (_pair_dinv(dc) if pair else np.ascontiguousarray(
                    np.broadcast_to(dc, (F, NB)))),
            }
        else:
            blk = adj[:, cs:cs + NB] * dinv[cs:cs + NB][None, :]
            blk[cs + idx, idx] += dinv[cs + idx]  # fold self-loop (+I)
            m = {
                "a": blk.astype(ml_dtypes.bfloat16),
                "z": z_dev,
                "bvec": b_dev,
            }
        in_maps.append(m)
    return in_maps


def _assemble(results, mode=None):
    """Device outputs -> full [N, F] output."""
    if mode is None:
        mode = MODE
    out = np.empty((N, F), dtype=np.float32)
    for c in range(NCORES):
        out[c * NB:(c + 1) * NB, :] = results[c]["o"].T
    return out


def kernel(x, adj, W, b):
    from concourse import bass_utils

    nc = _build_bass(mode=MODE)
    in_maps = _host_prep(x, adj, W, b, mode=MODE)
    res = bass_utils.run_bass_kernel_spmd(nc, in_maps,
                                          core_ids=list(range(NCORES)))
    return _assemble(res.results, mode=MODE)



# revision 65
# speedup vs baseline: 1.3369x; 1.1159x over previous
"""GCN layer kernel for 8 Trainium2 NeuronCores.

Computes: out = relu(dinv[:,None] * ((adj+I).T @ (dinv[:,None] * (x@W))) + b)
where dinv = rsqrt(colsum(adj) + 1).

Strategy (MODE="dr2"): shard adj by COLUMNS across the 8 cores. Column
block c (together with the full scaled source features z) is exactly what's
needed to produce output rows [c*2048, (c+1)*2048) -- no device collectives.
The roofline is the adjacency stream: 16384^2 entries at the 1 byte/elem
minimum the PE can consume (fp8e4m3, entries {0,1,2} exact with the
self-loop folded in) = 33.5 MB/core at ~360 GB/s/core HBM ~= 90 us.

The PE runs at 1 output-row/cycle regardless of dtype (measured), so with
only F=64 features a plain z^T A matmul wastes half the 128-wide array and
becomes the bottleneck (~107 us). Fix: fp8 DoubleRow matmuls (k=256) with
the stationary packed as [z8 | r8] across the full 128 columns, where
z8 = fp8(z * 2^9) and r8 = fp8(residual). One instruction per moving
A-pair then computes z8^T A into PSUM partitions 0:64 AND r8^T A into
64:128 from a single pass of the moving tile: 256 matmuls x 512 cycles =
54.6 us, fully hidden under the DMA stream. The residual recovers bf16-
class accuracy (rel err 2.9e-3 vs the 3.6e-2 of raw fp8 z).

Epilogue per core: ACT copies ps[64:128] to SBUF (bf16), 4 exact identity
matmuls fold it back into ps[0:64] (the PE is the only cross-partition
mover; this avoids burning DMA-highway bytes), DVE multiplies by
dinv * 2^-9, ACT applies relu+bias, and the [64, 2048] result is written
back as bf16.
"""

import sys

import numpy as np

if "/opt/trn_rl_repo" not in sys.path:
    sys.path.insert(0, "/opt/trn_rl_repo")

import ml_dtypes

N = 16384
F = 64
NCORES = 8
NB = N // NCORES  # 2048 columns (= output rows) per core
P = 128
KT = N // P  # 128 k-tiles of 128 source rows each
MM_N = 512  # moving-operand free dim per matmul (one PSUM bank of f32)
DMA_BATCH = 2  # k-tiles per dma_start
APOOL_BUFS = 14  # in-flight A-tile slots (prefetch depth)
ALT_DMA = False  # alternate A-tile DMAs between the SP and ACT HWDGE rings
A_PRETILED = False  # host pre-tiles A so each DMA descriptor is kb*2048B
NPACK = 28  # pk mode: tile-groups stored 2-entries/byte, decoded on ACT+DVE


NPACK2 = 32  # pk2: more packing, subs rotated across DVE and GPSIMD
NPACK3 = 33  # pk3: more packing, some b-slices decoded as Abs on ACT


NPACK4 = 34  # pk4: raw-v second slice, decode = one ACT Sign only


def _pk_npk(mode):
    return {"pk2": NPACK2, "pk3": NPACK3, "pk4": NPACK4}.get(mode, NPACK)


def _pk_abs(mode, i):
    """Always False: Abs-based b-decode is impossible — |v| encodes the
    product (2a-1)(2b-1), not b, so no pointwise one-op decode of the
    b-slice exists without the sign stream (the DVE sub). Kept so pk3
    degenerates to pk-with-NPACK3."""
    return False


def _pk_groups(ng, npk=None):
    """Evenly spread npk packed groups among ng so decode (ACT+DVE)
    overlaps the plain-group DMA stream instead of phase-serializing."""
    if npk is None:
        npk = NPACK
    return [g for g in range(ng)
            if (g * npk) // ng < ((g + 1) * npk) // ng]
ZSCALE_P = 9  # z is scaled by 2^ZSCALE_P before fp8 quantization (dr mode)
MODE = "pk4"  # variant kernel() uses

_BASS_CACHE: dict = {}


def _build_bass(reps: int = 1, mode: str = "full"):
    """Build the per-core Bass module. reps>1 repeats the whole compute
    (same inputs/outputs) inside one NEFF -- used only for benchmarking
    device time independent of dispatch overhead. mode: "full" | "dma"
    (loads only, no matmul) | "mm" (matmuls from a single resident tile,
    1/128th of the DMA traffic)."""
    key = (reps, mode, DMA_BATCH, APOOL_BUFS, ALT_DMA, A_PRETILED)
    if key in _BASS_CACHE:
        return _BASS_CACHE[key]

    import concourse.mybir as mybir
    import concourse.tile as tile
    from concourse import bacc

    if mode in ("dr", "drdma", "drmm"):
        nc = _build_bass_dr(reps, mode)
        _BASS_CACHE[key] = nc
        return nc
    if mode in ("dr2", "dr2dma", "dr2mm"):
        nc = _build_bass_dr2(reps, mode)
        _BASS_CACHE[key] = nc
        return nc
    if mode in ("pk", "pk2", "pk3", "pk4"):
        nc = _build_bass_pk(reps, mode)
        _BASS_CACHE[key] = nc
        return nc

    nc = bacc.Bacc("TRN2", target_bir_lowering=False, debug=False,
                   num_devices=NCORES)

    fp8 = mode in ("fp8", "fp8pair", "dma8", "mm8")
    pair = mode in ("pair", "fp8pair")
    a_dt = mybir.dt.float8e4 if fp8 else mybir.dt.bfloat16
    # pair mode: two col-group-tiled matmuls run concurrently. Each 512-chunk
    # nn gets its own PSUM bank (columns nn*512) with even chunks on
    # partitions 0-63 and odd chunks on 64-127, so no two accumulation
    # groups share a bank.
    b_p = 2 * F if pair else F       # bias/dinv partition count
    a_in = nc.dram_tensor("a", [N, NB], a_dt, kind="ExternalInput")
    z_in = nc.dram_tensor("z", [P, KT * F], mybir.dt.bfloat16,
                          kind="ExternalInput")
    b_in = nc.dram_tensor("bvec", [b_p, 1], mybir.dt.float32,
                          kind="ExternalInput")
    if fp8:
        d_in = nc.dram_tensor("dinv", [b_p, NB], mybir.dt.float32,
                              kind="ExternalInput")
    o_out = nc.dram_tensor("o", [F, NB], mybir.dt.float32,
                           kind="ExternalOutput")

    kb = DMA_BATCH
    # [KT/kb, 128, kb, NB]: group kb consecutive k-tiles into one DMA
    a_tiles = a_in.ap().rearrange("(g t p) i -> g p t i", t=kb, p=P)

    with tile.TileContext(nc) as tc:
        with (
            tc.tile_pool(name="singles", bufs=1) as singles,
            tc.tile_pool(name="apool", bufs=APOOL_BUFS) as apool,
            tc.tile_pool(name="psum", bufs=1, space="PSUM") as psum_pool,
        ):
            z_sb = singles.tile([P, KT * F], mybir.dt.bfloat16)
            nc.sync.dma_start(z_sb[:], z_in.ap())
            b_sb = singles.tile([b_p, 1], mybir.dt.float32)
            nc.sync.dma_start(b_sb[:], b_in.ap())
            d_sb = None
            if fp8:
                d_sb = singles.tile([b_p, NB], mybir.dt.float32, tag="d_sb")
                nc.sync.dma_start(d_sb[:], d_in.ap())

            mm_tile = None
            if mode in ("mm", "mm8"):
                mm_tile = singles.tile([P, kb, NB], a_dt, tag="mm_tile")
                nc.sync.dma_start(mm_tile[:], a_tiles[0])

            for _rep in range(reps):
                ps = psum_pool.tile([b_p, NB], mybir.dt.float32)

                for g in range(KT // kb):
                    if mode in ("mm", "mm8"):
                        at = mm_tile
                    else:
                        at = apool.tile([P, kb, NB], a_dt)
                        eng = nc.scalar if (ALT_DMA and g % 2) else nc.sync
                        eng.dma_start(at[:], a_tiles[g])
                    if mode in ("dma", "dma8"):
                        continue
                    for t in range(kb):
                        kt = g * kb + t
                        zk = z_sb[:, kt * F:(kt + 1) * F]
                        if pair:
                            for nn in range(NB // MM_N):
                                h = nn % 2
                                nc.tensor.matmul(
                                    ps[h * F:(h + 1) * F,
                                       nn * MM_N:(nn + 1) * MM_N],
                                    lhsT=zk,
                                    rhs=at[:, t, nn * MM_N:(nn + 1) * MM_N],
                                    start=(kt == 0),
                                    stop=(kt == KT - 1),
                                    tile_position=(0, h * F),
                                )
                        else:
                            for nn in range(NB // MM_N):
                                nc.tensor.matmul(
                                    ps[:, nn * MM_N:(nn + 1) * MM_N],
                                    lhsT=zk,
                                    rhs=at[:, t, nn * MM_N:(nn + 1) * MM_N],
                                    start=(kt == 0),
                                    stop=(kt == KT - 1),
                                )

                out_sb = singles.tile([b_p, NB], mybir.dt.float32,
                                      tag="out_sb")
                relu = mybir.ActivationFunctionType.Relu
                if mode in ("dma", "dma8"):
                    nc.vector.tensor_copy(out_sb[:F, :F], z_sb[:F, :F])
                    nc.sync.dma_start(o_out.ap(), out_sb[:F, :])
                elif pair:
                    # touch only the written PSUM quadrants
                    for nn in range(NB // MM_N):
                        h = nn % 2
                        sp = slice(h * F, (h + 1) * F)
                        sf = slice(nn * MM_N, (nn + 1) * MM_N)
                        if fp8:
                            nc.vector.tensor_mul(out_sb[sp, sf], ps[sp, sf],
                                                 d_sb[sp, sf])
                            nc.scalar.activation(out_sb[sp, sf],
                                                 out_sb[sp, sf], relu,
                                                 bias=b_sb[sp], scale=1.0)
                        else:
                            nc.scalar.activation(out_sb[sp, sf], ps[sp, sf],
                                                 relu, bias=b_sb[sp],
                                                 scale=1.0)
                        nc.sync.dma_start(o_out.ap()[:, sf], out_sb[sp, sf])
                elif fp8:
                    nc.vector.tensor_mul(out_sb[:], ps[:], d_sb[:])
                    nc.scalar.activation(out_sb[:], out_sb[:], relu,
                                         bias=b_sb[:], scale=1.0)
                    nc.sync.dma_start(o_out.ap(), out_sb[:])
                else:
                    nc.scalar.activation(out_sb[:], ps[:], relu,
                                         bias=b_sb[:], scale=1.0)
                    nc.sync.dma_start(o_out.ap(), out_sb[:])

    nc.compile()
    _BASS_CACHE[key] = nc
    return nc


def _build_bass_dr(reps: int = 1, mode: str = "dr"):
    """DoubleRow fp8 variant: both operands fp8e4m3, 0.5 cycles/row.

    z is split as z8 + r8 (fp8 value + fp8-quantized residual, both scaled
    by 2^ZSCALE_P); each A tile [128, 2, NB] is consumed by two DoubleRow
    matmuls per 512-chunk (z8 then r8) accumulating into the same PSUM
    region, so the PE does 2 x 256-deep contractions per tile at 0.5
    cycles/row = 54.6us/pass, fully hidden under the ~94us A-stream DMA.
    The self-loop (+I) stays folded into A; dinv and the 2^-ZSCALE_P are
    applied on PSUM readout. mode: "dr" | "drdma" (loads only) | "drmm"
    (matmuls from one resident tile).
    """
    import concourse.mybir as mybir
    import concourse.tile as tile
    from concourse import bacc

    nc = bacc.Bacc("TRN2", target_bir_lowering=False, debug=False,
                   num_devices=NCORES)

    kb = DMA_BATCH
    assert kb % 2 == 0
    a_dt = mybir.dt.float8e4
    dr = mybir.MatmulPerfMode.DoubleRow

    a_in = nc.dram_tensor("a", [N, NB], a_dt, kind="ExternalInput")
    z8_in = nc.dram_tensor("z8", [P, KT * F], a_dt, kind="ExternalInput")
    r8_in = nc.dram_tensor("r8", [P, KT * F], a_dt, kind="ExternalInput")
    b_in = nc.dram_tensor("bvec", [F, 1], mybir.dt.float32,
                          kind="ExternalInput")
    d_in = nc.dram_tensor("dinv", [F, NB], mybir.dt.float32,
                          kind="ExternalInput")
    o_out = nc.dram_tensor("o", [F, NB], mybir.dt.float32,
                           kind="ExternalOutput")

    # [KT/kb, 128, kb, NB]: group kb consecutive k-tiles into one DMA
    a_tiles = a_in.ap().rearrange("(g t p) i -> g p t i", t=kb, p=P)
    NG = KT // kb          # a-tile groups
    NPAIR = kb // 2        # DoubleRow k-pair matmul groups per a-tile
    NCHUNK = NB // MM_N    # 512-wide output chunks

    with tile.TileContext(nc) as tc:
        with (
            tc.tile_pool(name="singles", bufs=1) as singles,
            tc.tile_pool(name="apool", bufs=APOOL_BUFS) as apool,
            tc.tile_pool(name="psum", bufs=2, space="PSUM") as psum_pool,
        ):
            z8_sb = singles.tile([P, KT, F], a_dt, tag="z8_sb")
            nc.sync.dma_start(z8_sb[:], z8_in.ap())
            r8_sb = singles.tile([P, KT, F], a_dt, tag="r8_sb")
            nc.sync.dma_start(r8_sb[:], r8_in.ap())
            b_sb = singles.tile([F, 1], mybir.dt.float32, tag="b_sb")
            nc.sync.dma_start(b_sb[:], b_in.ap())
            d_sb = singles.tile([F, NB], mybir.dt.float32, tag="d_sb")
            nc.sync.dma_start(d_sb[:], d_in.ap())

            mm_tile = None
            if mode == "drmm":
                mm_tile = singles.tile([P, kb, NB], a_dt, tag="mm_tile")
                nc.sync.dma_start(mm_tile[:], a_tiles[0])

            for _rep in range(reps):
                ps = psum_pool.tile([F, NB], mybir.dt.float32)

                for g in range(NG):
                    if mode == "drmm":
                        at = mm_tile
                    else:
                        at = apool.tile([P, kb, NB], a_dt)
                        eng = nc.scalar if (ALT_DMA and g % 2) else nc.sync
                        eng.dma_start(at[:], a_tiles[g])
                    if mode == "drdma":
                        continue
                    for t2 in range(NPAIR):
                        kt0 = (g * kb + 2 * t2) if mode != "drmm" else 2 * t2
                        first = g == 0 and t2 == 0
                        last = g == NG - 1 and t2 == NPAIR - 1
                        for nn in range(NCHUNK):
                            sf = slice(nn * MM_N, (nn + 1) * MM_N)
                            nc.tensor.matmul(
                                ps[:, sf],
                                lhsT=z8_sb[:, kt0:kt0 + 2, :],
                                rhs=at[:, 2 * t2:2 * t2 + 2, sf],
                                start=first,
                                stop=False,
                                perf_mode=dr,
                            )
                            nc.tensor.matmul(
                                ps[:, sf],
                                lhsT=r8_sb[:, kt0:kt0 + 2, :],
                                rhs=at[:, 2 * t2:2 * t2 + 2, sf],
                                start=False,
                                stop=last,
                                perf_mode=dr,
                            )

                out_sb = singles.tile([F, NB], mybir.dt.float32,
                                      tag="out_sb")
                relu = mybir.ActivationFunctionType.Relu
                if mode == "drdma":
                    nc.vector.tensor_copy(out_sb[:, :F], z8_sb[:F, 0, :])
                    nc.sync.dma_start(o_out.ap(), out_sb[:])
                else:
                    nc.vector.tensor_mul(out_sb[:], ps[:], d_sb[:])
                    nc.scalar.activation(out_sb[:], out_sb[:], relu,
                                         bias=b_sb[:], scale=1.0)
                    nc.sync.dma_start(o_out.ap(), out_sb[:])

    nc.compile()
    return nc


def _build_bass_dr2(reps: int = 1, mode: str = "dr2"):
    """DoubleRow with the full 128-wide stationary: [z8 | r8] packed in m.

    The PE runs at 1 output-row/cycle regardless of dtype (measured), so
    the only way to cut cycles is to do more per row. One DoubleRow matmul
    (k=256, m=128, n=512) computes z8^T A into PSUM partitions 0:64 AND
    r8^T A into 64:128 from a single pass of the moving A-tile: 256
    matmuls x 512 cycles = 54.6us/pass, fully hidden under the ~94us
    A-stream DMA. Epilogue: DMA ps[64:128] back to partitions 0:64, DVE
    add + dinv mul, ACT relu+bias.
    """
    import concourse.mybir as mybir
    import concourse.tile as tile
    from concourse import bacc

    nc = bacc.Bacc("TRN2", target_bir_lowering=False, debug=False,
                   num_devices=NCORES)

    kb = DMA_BATCH
    assert kb % 2 == 0
    a_dt = mybir.dt.float8e4
    dr = mybir.MatmulPerfMode.DoubleRow

    if A_PRETILED:
        # host layout: row (g*128+p) holds the kb k-tile rows for partition
        # p of group g back to back -> one kb*2048B descriptor per partition
        a_in = nc.dram_tensor("a", [N // kb, kb * NB], a_dt,
                              kind="ExternalInput")
        a_tiles = a_in.ap().rearrange("(g p) (t i) -> g p t i", p=P, t=kb)
    else:
        a_in = nc.dram_tensor("a", [N, NB], a_dt, kind="ExternalInput")
        a_tiles = a_in.ap().rearrange("(g t p) i -> g p t i", t=kb, p=P)
    zr_in = nc.dram_tensor("zr", [P, KT * 2 * F], a_dt, kind="ExternalInput")
    b_in = nc.dram_tensor("bvec", [F, 1], mybir.dt.float32,
                          kind="ExternalInput")
    d_in = nc.dram_tensor("dinv", [F, NB], mybir.dt.bfloat16,
                          kind="ExternalInput")
    id_in = nc.dram_tensor("ident", [2 * F, F], mybir.dt.bfloat16,
                           kind="ExternalInput")
    o_out = nc.dram_tensor("o", [F, NB], mybir.dt.bfloat16,
                           kind="ExternalOutput")

    NG = KT // kb
    NPAIR = kb // 2
    NCHUNK = NB // MM_N

    with tile.TileContext(nc) as tc:
        with (
            tc.tile_pool(name="singles", bufs=1) as singles,
            tc.tile_pool(name="apool", bufs=APOOL_BUFS) as apool,
            tc.tile_pool(name="psum", bufs=2, space="PSUM") as psum_pool,
        ):
            zr_sb = singles.tile([P, KT, 2 * F], a_dt, tag="zr_sb")
            nc.sync.dma_start(zr_sb[:], zr_in.ap())
            b_sb = singles.tile([F, 1], mybir.dt.float32, tag="b_sb")
            nc.sync.dma_start(b_sb[:], b_in.ap())
            d_sb = singles.tile([F, NB], mybir.dt.bfloat16, tag="d_sb")
            nc.sync.dma_start(d_sb[:], d_in.ap())
            id_sb = singles.tile([2 * F, F], mybir.dt.bfloat16, tag="id_sb")
            nc.sync.dma_start(id_sb[:], id_in.ap())

            mm_tile = None
            if mode == "dr2mm":
                mm_tile = singles.tile([P, kb, NB], a_dt, tag="mm_tile")
                nc.sync.dma_start(mm_tile[:], a_tiles[0])

            for _rep in range(reps):
                ps = psum_pool.tile([2 * F, NB], mybir.dt.float32)

                for g in range(NG):
                    if mode == "dr2mm":
                        at = mm_tile
                    else:
                        at = apool.tile([P, kb, NB], a_dt)
                        eng = nc.scalar if (ALT_DMA and g % 2) else nc.sync
                        eng.dma_start(at[:], a_tiles[g])
                    if mode == "dr2dma":
                        continue
                    for t2 in range(NPAIR):
                        kt0 = (g * kb + 2 * t2) if mode != "dr2mm" else 2 * t2
                        first = g == 0 and t2 == 0
                        last = g == NG - 1 and t2 == NPAIR - 1
                        for nn in range(NCHUNK):
                            sf = slice(nn * MM_N, (nn + 1) * MM_N)
                            nc.tensor.matmul(
                                ps[:, sf],
                                lhsT=zr_sb[:, kt0:kt0 + 2, :],
                                rhs=at[:, 2 * t2:2 * t2 + 2, sf],
                                start=first,
                                stop=last,
                                perf_mode=dr,
                            )

                out_sb = singles.tile([F, NB], mybir.dt.bfloat16,
                                      tag="out_sb")
                relu = mybir.ActivationFunctionType.Relu
                if mode == "dr2dma":
                    nc.vector.tensor_copy(out_sb[:, :F], zr_sb[:F, 0, :F])
                    nc.scalar.dma_start(o_out.ap(), out_sb[:])
                else:
                    # fold ps[64:128] (r8 half) into ps[0:64] via the PE:
                    # ACT copies the hi half to SBUF (partition-aligned),
                    # then 4 exact f32 identity matmuls accumulate it into
                    # the lo-half PSUM region -- no DMA-highway traffic.
                    hi128 = singles.tile([2 * F, NB], mybir.dt.bfloat16,
                                         tag="hi128")
                    nc.scalar.copy(hi128[F:2 * F, :], ps[F:2 * F, :])
                    for nn in range(NCHUNK):
                        sf = slice(nn * MM_N, (nn + 1) * MM_N)
                        nc.tensor.matmul(
                            ps[:F, sf],
                            lhsT=id_sb[F:2 * F, :],
                            rhs=hi128[F:2 * F, sf],
                            start=False,
                            stop=True,
                            skip_group_check=True,
                        )
                    nc.vector.tensor_mul(out_sb[:], ps[:F, :], d_sb[:])
                    nc.scalar.activation(out_sb[:], out_sb[:], relu,
                                         bias=b_sb[:], scale=1.0)
                    nc.scalar.dma_start(o_out.ap(), out_sb[:])

    nc.compile()
    return nc


def _build_bass_pk(reps: int = 1, mode: str = "pk"):
    """dr2 + sign-domain packed A for NPACK of the 64 tile-groups.

    Packed byte v = (2a-1) + 0.125*(2b-1) (exact fp8) holds two k-rows per
    byte, halving HBM bytes for that fraction. Decode is one op per engine:
    sa = Sign(v) on ACT, t' = sa - v on DVE, written straight into the
    moving tile's two k-slices. The matmul consumes sa/t' with stationary
    slices pre-scaled (x0.5 for sign-slices, x-4 for t'-slices); the rank-1
    offsets (0.5 * colsum of z over packed rows) and the self-loop +z are
    folded into one zown tensor added to PSUM before the dinv multiply.
    NPACK balances DMA savings against ACT/DVE decode throughput.
    """
    import concourse.mybir as mybir
    import concourse.tile as tile
    from concourse import bacc

    nc = bacc.Bacc("TRN2", target_bir_lowering=False, debug=False,
                   num_devices=NCORES)

    kb = 2
    a_dt = mybir.dt.float8e4
    dr = mybir.MatmulPerfMode.DoubleRow
    NG = KT // kb
    NCHUNK = NB // MM_N
    npk = _pk_npk(mode)
    assert 0 <= npk <= NG
    pkset = set(_pk_groups(NG, npk))
    assert len(pkset) == npk

    a_in = nc.dram_tensor("a", [N - npk * 2 * P, NB], a_dt,
                          kind="ExternalInput")
    apk_in = nc.dram_tensor("apk", [npk * P, NB], a_dt,
                            kind="ExternalInput")
    zr_in = nc.dram_tensor("zr", [P, KT * 2 * F], a_dt, kind="ExternalInput")
    b_in = nc.dram_tensor("bvec", [F, 1], mybir.dt.float32,
                          kind="ExternalInput")
    d_in = nc.dram_tensor("dinv", [F, NB], mybir.dt.bfloat16,
                          kind="ExternalInput")
    zown_in = nc.dram_tensor("zown", [F, NB], mybir.dt.float32,
                             kind="ExternalInput")
    id_in = nc.dram_tensor("ident", [2 * F, F], mybir.dt.bfloat16,
                           kind="ExternalInput")
    o_out = nc.dram_tensor("o", [F, NB], mybir.dt.bfloat16,
                           kind="ExternalOutput")

    a_tiles = a_in.ap().rearrange("(g t p) i -> g p t i", t=kb, p=P)
    pk_tiles = apk_in.ap().rearrange("(g p) i -> g p i", p=P)

    with tile.TileContext(nc) as tc:
        with (
            tc.tile_pool(name="singles", bufs=1) as singles,
            tc.tile_pool(name="apool", bufs=APOOL_BUFS) as apool,
            tc.tile_pool(name="pkpool", bufs=6) as pkpool,
            tc.tile_pool(name="psum", bufs=2, space="PSUM") as psum_pool,
        ):
            zr_sb = singles.tile([P, KT, 2 * F], a_dt, tag="zr_sb")
            nc.sync.dma_start(zr_sb[:], zr_in.ap())
            b_sb = singles.tile([F, 1], mybir.dt.float32, tag="b_sb")
            nc.sync.dma_start(b_sb[:], b_in.ap())
            d_sb = singles.tile([F, NB], mybir.dt.bfloat16, tag="d_sb")
            nc.sync.dma_start(d_sb[:], d_in.ap())
            zown_sb = singles.tile([F, NB], mybir.dt.float32, tag="zown_sb")
            nc.sync.dma_start(zown_sb[:], zown_in.ap())
            id_sb = singles.tile([2 * F, F], mybir.dt.bfloat16, tag="id_sb")
            nc.sync.dma_start(id_sb[:], id_in.ap())

            for _rep in range(reps):
                ps = psum_pool.tile([2 * F, NB], mybir.dt.float32)

                pk_idx = 0
                pl_idx = 0
                for g in range(NG):
                    at = apool.tile([P, kb, NB], a_dt)
                    if g in pkset and mode == "pk4":
                        # raw v IS slice 1; decode slice 0 = Sign(v) in place
                        nc.sync.dma_start(at[:, 1, :], pk_tiles[pk_idx])
                        pk_idx += 1
                        nc.scalar.sign(at[:, 0, :], at[:, 1, :])
                    elif g in pkset:
                        ip = pk_idx
                        vt = pkpool.tile([P, NB], a_dt)
                        nc.sync.dma_start(vt[:], pk_tiles[ip])
                        pk_idx += 1
                        nc.scalar.sign(at[:, 0, :], vt[:])
                        if _pk_abs(mode, ip):
                            nc.scalar.activation(
                                at[:, 1, :], vt[:],
                                mybir.ActivationFunctionType.Abs)
                        else:
                            sub_eng = (nc.gpsimd if (mode == "pk2" and
                                                     ip % 3 == 2)
                                       else nc.vector)
                            sub_eng.tensor_sub(at[:, 1, :], at[:, 0, :],
                                               vt[:])
                    else:
                        nc.sync.dma_start(at[:], a_tiles[pl_idx])
                        pl_idx += 1
                    kt0 = 2 * g
                    for nn in range(NCHUNK):
                        sf = slice(nn * MM_N, (nn + 1) * MM_N)
                        nc.tensor.matmul(
                            ps[:, sf],
                            lhsT=zr_sb[:, kt0:kt0 + 2, :],
                            rhs=at[:, :, sf],
                            start=(g == 0),
                            stop=(g == NG - 1),
                            perf_mode=dr,
                        )

                out_sb = singles.tile([F, NB], mybir.dt.bfloat16,
                                      tag="out_sb")
                relu = mybir.ActivationFunctionType.Relu
                hi128 = singles.tile([2 * F, NB], mybir.dt.bfloat16,
                                     tag="hi128")
                nc.scalar.copy(hi128[F:2 * F, :], ps[F:2 * F, :])
                for nn in range(NCHUNK):
                    sf = slice(nn * MM_N, (nn + 1) * MM_N)
                    nc.tensor.matmul(
                        ps[:F, sf],
                        lhsT=id_sb[F:2 * F, :],
                        rhs=hi128[F:2 * F, sf],
                        start=False,
                        stop=True,
                        skip_group_check=True,
                    )
                tmp_sb = singles.tile([F, NB], mybir.dt.float32,
                                      tag="tmp_sb")
                nc.vector.tensor_add(tmp_sb[:], ps[:F, :], zown_sb[:])
                nc.vector.tensor_mul(out_sb[:], tmp_sb[:], d_sb[:])
                nc.scalar.activation(out_sb[:], out_sb[:], relu,
                                     bias=b_sb[:], scale=1.0)
                nc.scalar.dma_start(o_out.ap(), out_sb[:])

    nc.compile()
    return nc


def _host_prep(x, adj, W, b, mode=None):
    """Host-side sharding/preprocessing -> per-core input maps."""
    if mode is None:
        mode = MODE
    fp8 = mode in ("fp8", "fp8pair", "dma8", "mm8")
    pair = mode in ("pair", "fp8pair")
    x = np.asarray(x, dtype=np.float32)
    adj = np.asarray(adj, dtype=np.float32)
    W = np.asarray(W, dtype=np.float32)
    b = np.asarray(b, dtype=np.float32)

    deg = adj.sum(axis=0) + 1.0
    dinv = np.where(deg > 0, 1.0 / np.sqrt(deg), 0.0).astype(np.float32)

    z = (dinv[:, None] * (x @ W)).astype(np.float32)  # [N, F]

    if mode in ("pk", "pk2", "pk3", "pk4"):
        fp8 = ml_dtypes.float8_e4m3
        s = np.float32(2.0 ** ZSCALE_P)
        zs = z * s
        z8 = zs.astype(fp8)
        z8f = z8.astype(np.float32)
        r8 = (zs - z8f).astype(fp8)
        r8f = r8.astype(np.float32)
        npk = _pk_npk(mode)
        pklist = _pk_groups(KT // 2, npk)
        pkset = set(pklist)
        pllist = [g for g in range(KT // 2) if g not in pkset]

        fac = np.ones(KT, np.float32)
        boff = np.zeros(KT, np.float32)  # corr weight for b-rows
        for i, g in enumerate(pklist):
            if mode == "pk4":
                boff[2 * g] = boff[2 * g + 1] = 0.5
                continue  # packed zr slices overwritten with W-pairs below
            fac[2 * g] = 0.5
            boff[2 * g] = 0.5
            if _pk_abs(mode, i):
                fac[2 * g + 1] = 4.0
                boff[2 * g + 1] = -3.5
            else:
                fac[2 * g + 1] = -4.0
                boff[2 * g + 1] = 0.5
        rowfac = np.repeat(fac, P)[:, None]

        def _kmajor(m):
            return np.ascontiguousarray(
                m.reshape(KT, P, F).transpose(1, 0, 2))

        zr = np.concatenate(
            [_kmajor((z8f * rowfac).astype(fp8)),
             _kmajor((r8f * rowfac).astype(fp8))], axis=2)
        zsum = z8f + r8f
        if mode == "pk4":
            # packed stationaries carry stream-combos: sa-slice W0 =
            # 0.5 z'a - 4 z'b, v-slice W1 = 4 z'b, each as fp8 + residual
            for g in pklist:
                za = zs[g * 2 * P:g * 2 * P + P]
                zb = zs[g * 2 * P + P:(g + 1) * 2 * P]
                for kt, W in ((2 * g, 0.5 * za - 4.0 * zb),
                              (2 * g + 1, 4.0 * zb)):
                    q = W.astype(fp8)
                    zr[:, kt, :F] = q
                    zr[:, kt, F:] = (W - q.astype(np.float32)).astype(fp8)
            roww = np.repeat(boff, P)[:, None].astype(np.float64)
            corr = (roww * zs.astype(np.float64)).sum(
                axis=0).astype(np.float32)  # exact z' for packed rows
        else:
            roww = np.repeat(boff, P)[:, None].astype(np.float64)
            corr = (roww * zsum.astype(np.float64)).sum(
                axis=0).astype(np.float32)  # [F]
        zr_dev = np.ascontiguousarray(zr.reshape(P, KT * 2 * F))
        b_dev = np.ascontiguousarray(b.reshape(F, 1))
        ident = np.zeros((2 * F, F), ml_dtypes.bfloat16)
        ident[F + np.arange(F), np.arange(F)] = 1.0
        in_maps = []
        for c in range(NCORES):
            cs = c * NB
            blk = adj[:, cs:cs + NB]  # raw {0,1}; self-loop via zown
            A4 = blk.reshape(KT // 2, 2, P, NB)
            Apk = A4[pklist]  # [npk, 2, P, NB]
            v = (2.0 * Apk[:, 0] - 1.0) + 0.125 * (2.0 * Apk[:, 1] - 1.0)
            Apl = A4[pllist].reshape(-1, NB)  # plain groups, (g t p) order
            dc = (dinv[cs:cs + NB] / s).astype(np.float32)
            zown = zsum[cs:cs + NB, :].T + corr[:, None]  # [F, NB]
            in_maps.append({
                "a": np.ascontiguousarray(Apl).astype(fp8),
                "apk": np.ascontiguousarray(
                    v.reshape(npk * P, NB)).astype(fp8),
                "zr": zr_dev,
                "bvec": b_dev,
                "dinv": np.ascontiguousarray(
                    np.broadcast_to(dc, (F, NB)).astype(ml_dtypes.bfloat16)),
                "zown": np.ascontiguousarray(zown.astype(np.float32)),
                "ident": ident,
            })
        return in_maps

    if mode in ("dr", "drdma", "drmm", "dr2", "dr2dma", "dr2mm"):
        fp8 = ml_dtypes.float8_e4m3
        s = np.float32(2.0 ** ZSCALE_P)
        zs = z * s
        z8 = zs.astype(fp8)
        r8 = (zs - z8.astype(np.float32)).astype(fp8)

        def _kmajor(m):
            return np.ascontiguousarray(
                m.reshape(KT, P, F).transpose(1, 0, 2))  # [P, KT, F]

        z8_km = _kmajor(z8)
        r8_km = _kmajor(r8)
        b_dev = np.ascontiguousarray(b.reshape(F, 1))
        idx = np.arange(NB)
        in_maps = []
        for c in range(NCORES):
            cs = c * NB
            blk = adj[:, cs:cs + NB].copy()
            blk[cs + idx, idx] += 1.0  # self-loop (+I), exact in fp8
            dc = (dinv[cs:cs + NB] / s).astype(np.float32)
            d_dt = ml_dtypes.bfloat16 if mode.startswith("dr2") else np.float32
            a_dev = blk.astype(fp8)
            if mode.startswith("dr2") and A_PRETILED:
                kb = DMA_BATCH
                a_dev = np.ascontiguousarray(
                    a_dev.reshape(KT // kb, kb, P, NB)
                    .transpose(0, 2, 1, 3).reshape(N // kb, kb * NB))
            m = {
                "a": a_dev,
                "bvec": b_dev,
                "dinv": np.ascontiguousarray(
                    np.broadcast_to(dc, (F, NB)).astype(d_dt)),
            }
            if mode.startswith("dr2"):
                zr = np.concatenate([z8_km, r8_km], axis=2)  # [P, KT, 2F]
                m["zr"] = np.ascontiguousarray(zr.reshape(P, KT * 2 * F))
                ident = np.zeros((2 * F, F), ml_dtypes.bfloat16)
                ident[F + np.arange(F), np.arange(F)] = 1.0
                m["ident"] = ident
            else:
                m["z8"] = np.ascontiguousarray(z8_km.reshape(P, KT * F))
                m["r8"] = np.ascontiguousarray(r8_km.reshape(P, KT * F))
            in_maps.append(m)
        return in_maps
    # k-major layout: z_sb[p, kt*F + f] = z[kt*128 + p, f]
    z_dev = np.ascontiguousarray(
        z.reshape(KT, P, F).transpose(1, 0, 2).reshape(P, KT * F)
    ).astype(ml_dtypes.bfloat16)

    if pair:
        b_dev = np.ascontiguousarray(
            np.concatenate([b, b]).reshape(2 * F, 1))
    else:
        b_dev = np.ascontiguousarray(b.reshape(F, 1))

    def _pair_dinv(dc):
        # [128, NB]: chunk nn lives at [64*(nn%2):64*(nn%2+1), nn*512:...]
        d = np.zeros((2 * F, NB), np.float32)
        for nn in range(NB // MM_N):
            h = nn % 2
            d[h * F:(h + 1) * F, nn * MM_N:(nn + 1) * MM_N] = \
                dc[nn * MM_N:(nn + 1) * MM_N]
        return d

    in_maps = []
    idx = np.arange(NB)
    for c in range(NCORES):
        cs = c * NB
        if fp8:
            # adjacency stays exact {0,1,2} in fp8; dinv applied on device
            blk = adj[:, cs:cs + NB].copy()
            blk[cs + idx, idx] += 1.0  # self-loop (+I)
            dc = dinv[cs:cs + NB]
            m = {
                "a": blk.astype(ml_dtypes.float8_e4m3),
                "z": z_dev,
                "bvec": b_dev,
                "dinv": (_pair_dinv(dc) if pair else np.ascontiguousarray(
                    np.broadcast_to(dc, (F, NB)))),
            }
        else:
            blk = adj[:, cs:cs + NB] * dinv[cs:cs + NB][None, :]
            blk[cs + idx, idx] += dinv[cs + idx]  # fold self-loop (+I)
            m = {
                "a": blk.astype(ml_dtypes.bfloat16),
                "z": z_dev,
                "bvec": b_dev,
            }
        in_maps.append(m)
    return in_maps


def _assemble(results, mode=None):
    """Device outputs -> full [N, F] output."""
    if mode is None:
        mode = MODE
    out = np.empty((N, F), dtype=np.float32)
    for c in range(NCORES):
        out[c * NB:(c + 1) * NB, :] = results[c]["o"].T
    return out


def kernel(x, adj, W, b):
    from concourse import bass_utils

    nc = _build_bass(mode=MODE)
    in_maps = _host_prep(x, adj, W, b, mode=MODE)
    res = bass_utils.run_bass_kernel_spmd(nc, in_maps,
                                          core_ids=list(range(NCORES)))
    return _assemble(res.results, mode=MODE)

